# revision 21
# baseline (speedup 1.0000x reference)
"""Trainium2 Bass kernel for nn_CapsGATattentionGRU (B=128, T=32, D=32, H=64, F=2048).

Sharding: GRU recurrence replicated on 8 cores (whh0 fp8 + whh1 fp16 SBUF-resident,
col-tiled packed matmuls, DVE block-transpose feedback); x-side gate inputs
N-sharded + AllGathered; layer-1 x-side matmuls merged into recurrence-0's
step boundaries (consuming SBUF-resident hT tiles); downstream batch-sharded
via one-hot gather matmul.
"""
import os, sys, time
sys.path.insert(0, '/opt/trn_rl_repo')
import numpy as np
import ml_dtypes

import concourse.bass as bass
import concourse.bacc as bacc
import concourse.tile as tile
from concourse import mybir
from concourse.bass_utils import run_bass_kernel_spmd

f8 = mybir.dt.float8e4
f16 = mybir.dt.float16
f32 = mybir.dt.float32
AF = mybir.ActivationFunctionType

D, T, H_, B = 32, 32, 64, 128
F = D * H_
KT = 16
NC = 8
SH = B // NC
S = 32.0   # fp8 weight prescale; ACT un-scale 1/S
DEBUG = os.environ.get("KBUILD_DEBUG", "") == "1"


def hd_perm():
    out = np.zeros(2048, np.int64)
    for Hh in range(2):
        for j in range(4):
            hds = Hh*1024 + (np.arange(8)[:, None]*128 + j*32 + np.arange(32)[None, :]).reshape(-1)
            out[(Hh*4+j)*256:(Hh*4+j)*256+256] = hds
    return out

PERM = hd_perm()


def _gate_cols(whmat):
    """whmat (6144, K) -> (K, 6144) transposed with perm'd col order."""
    K = whmat.shape[1]
    out = np.zeros((K, 6144), np.float32)
    for q in range(8):
        hds = PERM[q*256:(q+1)*256]
        for c in range(3):
            out[:, q*768 + c*256: q*768 + (c+1)*256] = whmat[c*2048 + hds].T
    return out


def build_program():
    nc = bacc.Bacc("TRN2", target_bir_lowering=False, debug=False, num_devices=NC)

    xT_d = nc.dram_tensor("xT", [B, 128, KT*32], f16, kind="ExternalInput")
    wih0_d = nc.dram_tensor("wih0", [KT, 128, 768], f16, kind="ExternalInput")
    wih1_d = nc.dram_tensor("wih1", [KT, 128, 768], f8, kind="ExternalInput")
    b0_d = nc.dram_tensor("b0", [1, 768], f16, kind="ExternalInput")
    b1_d = nc.dram_tensor("b1", [1, 768], f16, kind="ExternalInput")
    whh0_d = nc.dram_tensor("whh0", [KT, 128, 6144], f8, kind="ExternalInput")
    whh1_d = nc.dram_tensor("whh1", [KT, 128, 6144], f8, kind="ExternalInput")
    bhn0_d = nc.dram_tensor("bhn0", [128, 2, 256], f16, kind="ExternalInput")
    bhn1_d = nc.dram_tensor("bhn1", [128, 2, 256], f16, kind="ExternalInput")
    eye_d = nc.dram_tensor("eye", [128, 128], f16, kind="ExternalInput")
    sel_d = nc.dram_tensor("sel", [128, SH], f16, kind="ExternalInput")
    awt_d = nc.dram_tensor("awt", [128, 32], f16, kind="ExternalInput")
    ab_d = nc.dram_tensor("ab", [128, 1], f32, kind="ExternalInput")
    od_d = nc.dram_tensor("od", [128, 4], f16, kind="ExternalInput")
    od2_d = nc.dram_tensor("od2", [4, 128], f16, kind="ExternalInput")
    gw_d = nc.dram_tensor("gw", [65, 4, 64], f16, kind="ExternalInput")
    gatt_d = nc.dram_tensor("gat_att", [2, 128, 64], f16, kind="ExternalInput")
    gbias_d = nc.dram_tensor("gat_bias", [2, 128, 64], f16, kind="ExternalInput")
    wc_d = nc.dram_tensor("wc", [16, 128, 128], f16, kind="ExternalInput")
    fw_d = nc.dram_tensor("fw", [65, 32], f16, kind="ExternalInput")

    out_d = nc.dram_tensor("out", [32, 32, SH], f16, kind="ExternalOutput")
    if DEBUG:
        dbg_emb = nc.dram_tensor("dbg_emb", [SH, 32, 2048], f16, kind="ExternalOutput")
        dbg_attv = nc.dram_tensor("dbg_attv", [SH, 2048], f16, kind="ExternalOutput")
        dbg_g01 = nc.dram_tensor("dbg_g01", [2, 128, 64], f16, kind="ExternalOutput")
        dbg_caps = nc.dram_tensor("dbg_caps", [16, 128, 16], f16, kind="ExternalOutput")

    NCHUNK = 4
    CS = B // NCHUNK   # steps per AG chunk
    ag0_in = [nc.dram_tensor(f"ag0_in{j}", [CS, 32, 768], f16) for j in range(NCHUNK)]
    ag0_out = [nc.dram_tensor(f"ag0_out{j}", [NC, CS, 32, 768], f16, addr_space="Shared")
               for j in range(NCHUNK)]
    ag1_in = [nc.dram_tensor(f"ag1_in{j}", [CS, 32, 768], f16) for j in range(NCHUNK)]
    ag1_out = [nc.dram_tensor(f"ag1_out{j}", [NC, CS, 32, 768], f16, addr_space="Shared")
               for j in range(NCHUNK)]
    warm_in = nc.dram_tensor("warm_in", [1, 16], f16)
    warm_out = nc.dram_tensor("warm_out", [NC, 1, 16], f16, addr_space="Shared")
    hnat = nc.dram_tensor("hnat", [B, 32, 2048], f16)
    emb_mine = nc.dram_tensor("emb_mine", [SH, 32, 2048], f16)
    att_pad = nc.dram_tensor("att_pad", [512, 128], f16)
    fus_nat = nc.dram_tensor("fus_nat", [512, 128], f16)
    caps_pad = nc.dram_tensor("caps_pad", [128, 512], f16)

    with tile.TileContext(nc) as tc:
        ctxs = []
        def pool(**kw):
            p = tc.tile_pool(**kw)
            ctxs.append(p)
            return p.__enter__()
        wp = pool(name="wp", bufs=1)
        sb = pool(name="sb", bufs=1)
        gip = pool(name="gip", bufs=2)
        hp = pool(name="hp", bufs=2)
        psp = pool(name="ps", bufs=2, space="PSUM")

        # ---- psum tag rotation for downstream: 3 tags x 2 bufs (+px = 8 banks) ----
        ps_ctr = [0]
        ps_tags = ["ps0", "ps1", "ps2", "px"]
        ps_mod = [3]
        def ps_tile(shape, name):
            tag = ps_tags[ps_ctr[0] % ps_mod[0]]
            ps_ctr[0] += 1
            return psp.tile(shape, f32, name=name, tag=tag)

        # big-slot helper: one ~192KB slot (tag "big"), carved manually
        def big_tile(name, cols, dt=f16):
            return wp.tile([128, cols], dt, name=name, tag="big")

        # epoch0: phase_x0 f16 workspace + recurrence0 fp8 weights in ONE slot so
        # the whh0/wih1 loads overlap px0 compute.
        W0, W1 = KT*6144, KT*768
        E0_PX = W0 + W1 + 1536 + 2*768 + 5*512   # fp8-col offset of px0 region

        # ================= phase A: x-side of layer 0 =================
        def phase_x0(w):
            # layout (f16 cols): wih 12288 | xt 8x512 | bias 768 | bbb 768 | gio 6x384
            wih = w[:, 0:12288].rearrange("p (k n) -> p k n", k=KT)
            for k in range(KT):
                nc.sync.dma_start(wih[:, k, :], wih0_d[k])
            xts_ab = [[w[:, 12288+512*(4*ab_+s): 12288+512*(4*ab_+s+1)].rearrange("p (k b) -> p k b", k=KT)
                       for s in range(4)] for ab_ in range(2)]
            bb = w[0:1, 16384:17152]
            nc.sync.dma_start(bb, b0_d[:, :])
            bbb = w[:, 17152:17920]
            ones1 = sb.tile([1, 128], f16, name="ones1", tag="ones1")
            nc.vector.memset(ones1[:], 1.0)
            for half in range(2):
                pb = ps_tile([128, 384], f"pb{half}")
                nc.tensor.matmul(out=pb[:], lhsT=ones1[:], rhs=bb[:, 384*half:384*half+384],
                                 start=True, stop=True)
                nc.vector.tensor_copy(bbb[:, 384*half:384*half+384], pb[:])
            gio_off = 17920
            for p in range(B // 4):
                xts = xts_ab[p % 2]
                for s in range(4):
                    t = p*4 + s
                    eng = nc.sync if s % 2 == 0 else nc.scalar
                    eng.dma_start(xts[s][:], xT_d[t].rearrange("p (k b) -> p k b", k=KT))
                jc = (p * 4) // CS
                pl = p * 4 - jc * CS
                for ch in range(2):
                    ps = ps_tile([128, 384], f"psA{p}{ch}")
                    for k in range(KT):
                        for s in range(4):
                            nc.tensor.matmul(
                                out=ps[32*s:32*s+32, :],
                                lhsT=xts[s][:, k, :],
                                rhs=wih[:, k, 384*ch:384*ch+384],
                                start=(k == 0), stop=(k == KT-1),
                                tile_position=(0, 32*s))
                    gio = w[:, gio_off + ((p % 3)*2 + ch)*384: gio_off + ((p % 3)*2 + ch + 1)*384]
                    nc.vector.tensor_add(gio, ps[:], bbb[:, 384*ch:384*ch+384])
                    nc.scalar.dma_start(
                        out=ag0_in[jc][pl:pl+4, :, 384*ch:384*ch+384].rearrange("s b n -> (s b) n"),
                        in_=gio)
                if (p * 4) % CS == CS - 4:
                    nc.gpsimd.collective_compute(
                        "AllGather", mybir.AluOpType.bypass,
                        replica_groups=[list(range(NC))],
                        ins=[ag0_in[jc].ap().opt()], outs=[ag0_out[jc].ap().opt()])

        # ================= recurrence =================
        def recurrence0(w):
            """Layer-0 recurrence (fp8 whh) with layer-1 x-side (fp8 wih1)
            merged at 4-step boundaries; h fed back via DVE transpose + fp8 cast."""
            # carve (f8 cols): whh0 | wih1 | bbb1(f16) | gio1 x2 (f16) | hT8 ring 5x512
            whh = w[:, 0:W0].rearrange("p (k n) -> p k n", k=KT)
            for k in range(KT):
                eng = nc.sync if k % 2 == 0 else nc.scalar
                eng.dma_start(whh[:, k, :], whh0_d[k])
            wih1 = w[:, W0:W0+W1].rearrange("p (k n) -> p k n", k=KT)
            for k in range(KT):
                eng = nc.scalar if k % 2 == 0 else nc.sync
                eng.dma_start(wih1[:, k, :], wih1_d[k])
            bbb1 = w[:, W0+W1:W0+W1+1536].bitcast(f16)
            gio1 = [w[:, W0+W1+1536:W0+W1+2560].bitcast(f16),
                    w[:, W0+W1+2560:W0+W1+3072].bitcast(f16)]
            hT8ring = [w[:, W0+W1+3072+512*i:W0+W1+3072+512*(i+1)].rearrange(
                       "p (h n) -> p h n", h=2) for i in range(5)]
            # bbb1 broadcast via ones matmul
            bb1 = w[:, E0_PX + 2*20224: E0_PX + 2*20224 + 1536].bitcast(f16)[0:1, :]
            nc.sync.dma_start(bb1, b1_d[:, :])
            ones1 = sb.tile([1, 128], f16, name="ones1b", tag="ones1")
            nc.vector.memset(ones1[:], 1.0)
            for half in range(2):
                pb = psp.tile([128, 384], f32, name=f"pb1{half}", tag="px")
                nc.tensor.matmul(out=pb[:], lhsT=ones1[:], rhs=bb1[:, 384*half:384*half+384],
                                 start=True, stop=True)
                nc.vector.tensor_copy(bbb1[:, 384*half:384*half+384], pb[:])

            eye = sb.tile([128, 128], f16, name="eye0", tag="eye")
            nc.sync.dma_start(eye[:], eye_d[:, :])
            bhn = sb.tile([128, 2, 256], f16, name="bhn0", tag="bhn")
            nc.sync.dma_start(bhn[:], bhn0_d[:, :, :])
            wtile = sb.tile([1, 16], f16, name="wtile", tag="wtile")
            nc.scalar.dma_start(wtile[:], warm_out[0])
            nc.vector.tensor_add(bhn[0:1, 0, 0:16], bhn[0:1, 0, 0:16], wtile[:])
            hT = [hT8ring[0][:, Hh, :] for Hh in range(2)]
            hg = [hp.tile([128, 256], f16, name=f"hg{Hh}", tag=f"hg{Hh}", bufs=2) for Hh in range(2)]
            for Hh in range(2):
                nc.vector.memset(hT[Hh][:], 0.0)
                nc.vector.memset(hg[Hh][:], 0.0)
            hist = []

            def emit_px1(p):
                # layer-1 x-side for steps 4p..4p+3 from SBUF hT8 history
                jc = (p * 4) // CS
                pl = p * 4 - jc * CS
                for ch, (c0, cw) in enumerate(((0, 512), (512, 256))):
                    ps = psp.tile([128, cw], f32, name=f"px{p}{ch}", tag="px")
                    for k in range(KT):
                        kp, kk = k // 8, k % 8
                        for s in range(4):
                            nc.tensor.matmul(
                                out=ps[32*s:32*s+32, :],
                                lhsT=hist[4*p+s][kp][:, 32*kk:32*kk+32],
                                rhs=wih1[:, k, c0:c0+cw],
                                start=(k == 0), stop=(k == KT-1),
                                tile_position=(0, 32*s))
                    gio = gio1[ch][:, 0:cw]
                    nc.vector.tensor_add(gio, ps[:], bbb1[:, c0:c0+cw])
                    nc.scalar.dma_start(
                        out=ag1_in[jc][pl:pl+4, :, c0:c0+cw].rearrange("s b n -> (s b) n"),
                        in_=gio)
                if (p * 4) % CS == CS - 4:
                    nc.gpsimd.collective_compute(
                        "AllGather", mybir.AluOpType.bypass,
                        replica_groups=[list(range(NC))],
                        ins=[ag1_in[jc].ap().opt()], outs=[ag1_out[jc].ap().opt()])

            for t in range(B):
                jc, tl = t // CS, t % CS
                newhT = [None, None]
                newhg = [None, None]
                pa, gis = [], []
                pn = [psp.tile([128, 256], f32, name=f"pn{t}{Hh}", tag="ps2")
                      for Hh in range(2)]
                for Hh in range(2):
                    pa.append(psp.tile([128, 512], f32, name=f"pa{t}{Hh}", tag=f"ps{Hh}"))
                    gi_sb = gip.tile([128, 3, 256], f16, name=f"gi{t}_{Hh}", tag="gi")
                    nc.sync.dma_start(
                        gi_sb[:],
                        ag0_out[jc][Hh*4:Hh*4+4, tl].rearrange("s b (c n) -> s b c n", c=3))
                    gis.append(gi_sb)
                def kgroup(kp, Hh):
                    for kk in range(8):
                        k = 8*kp + kk
                        lhsT = hT[kp][:, 32*kk:32*kk+32]
                        for j in range(4):
                            base = (Hh*4+j)*768
                            nc.tensor.matmul(out=pa[Hh][32*j:32*j+32, :], lhsT=lhsT,
                                rhs=whh[:, k, base:base+512],
                                start=(k == 0), stop=False, tile_position=(0, 32*j))
                        for j in range(4):
                            base = (Hh*4+j)*768
                            nc.tensor.matmul(out=pn[Hh][32*j:32*j+32, :], lhsT=lhsT,
                                rhs=whh[:, k, base+512:base+768],
                                start=(k == 0), stop=False, tile_position=(0, 32*j))
                kgroup(0, 0)
                kgroup(0, 1)
                kgroup(1, 0)
                for Hh in range(2):
                    if Hh == 1:
                        kgroup(1, 1)
                    gi_sb = gis[Hh]
                    nc.tensor.matmul(out=pa[Hh][:], lhsT=eye[:],
                                     rhs=gi_sb[:, 0:2, :].rearrange("p c n -> p (c n)"),
                                     start=False, stop=True)
                    nc.tensor.matmul(out=pn[Hh][:], lhsT=eye[:], rhs=bhn[:, Hh, :],
                                     start=False, stop=True)
                    rz = sb.tile([128, 512], f16, name=f"rz{t}{Hh}", tag=f"rz{Hh}", bufs=1)
                    nc.scalar.activation(rz[:], pa[Hh][:], AF.Sigmoid, scale=1.0/S)
                    r = rz[:, 0:256]
                    z = rz[:, 256:512]
                    tn = sb.tile([128, 256], f16, name=f"tn{t}{Hh}", tag="gt", bufs=3)
                    nc.vector.tensor_mul(tn[:], r, pn[Hh][:])
                    tn2 = sb.tile([128, 256], f16, name=f"tn2{t}{Hh}", tag="gt", bufs=3)
                    nc.vector.tensor_add(tn2[:], tn[:], gi_sb[:, 2, :])
                    n_ = sb.tile([128, 256], f16, name=f"n{t}{Hh}", tag=f"n{Hh}", bufs=1)
                    nc.scalar.activation(n_[:], tn2[:], AF.Tanh, scale=1.0/S)
                    d_ = sb.tile([128, 256], f16, name=f"d{t}{Hh}", tag="gt", bufs=3)
                    nc.vector.tensor_sub(d_[:], hg[Hh][:], n_[:])
                    zd = sb.tile([128, 256], f16, name=f"zd{t}{Hh}", tag=f"zd{Hh}", bufs=1)
                    nc.vector.tensor_mul(zd[:], z, d_[:])
                    hn = hp.tile([128, 256], f16, name=f"hg{Hh}", tag=f"hg{Hh}")
                    nc.vector.tensor_add(hn[:], n_[:], zd[:])
                    newhg[Hh] = hn
                    nhT = hp.tile([128, 256], f16, name=f"hT{Hh}", tag=f"hT{Hh}", bufs=2)
                    nc.vector.transpose(nhT[:], hn[:])
                    nhT8 = hT8ring[(t + 1) % 5][:, Hh, :]
                    nc.vector.tensor_copy(nhT8, nhT[:])
                    newhT[Hh] = nhT8
                hT = newhT
                hg = newhg
                hist.append((newhT[0], newhT[1]))
                if t % 4 == 0 and t >= 4:
                    emit_px1(t // 4 - 1)
            emit_px1(31)

        def recurrence1():
            whh = wp.tile([128, KT, 6144], f8, name="whh1", tag="big")
            for k in range(KT):
                eng = nc.sync if k % 2 == 0 else nc.scalar
                eng.dma_start(whh[:, k, :], whh1_d[k])
            eye = sb.tile([128, 128], f16, name="eye1", tag="eye")
            nc.sync.dma_start(eye[:], eye_d[:, :])
            bhn = sb.tile([128, 2, 256], f16, name="bhn1", tag="bhn")
            nc.sync.dma_start(bhn[:], bhn1_d[:, :, :])
            hT = [hp.tile([128, 256], f8, name=f"hT8b{Hh}", tag=f"hT8b{Hh}", bufs=2)
                  for Hh in range(2)]
            hg = [hp.tile([128, 256], f16, name=f"hg{Hh}", tag=f"hg{Hh}", bufs=2) for Hh in range(2)]
            for Hh in range(2):
                nc.vector.memset(hT[Hh][:], 0.0)
                nc.vector.memset(hg[Hh][:], 0.0)
            for t in range(B):
                jc, tl = t // CS, t % CS
                newhT = [None, None]
                newhg = [None, None]
                pa, gis = [], []
                pn = [psp.tile([128, 256], f32, name=f"pn1{t}{Hh}", tag="ps2")
                      for Hh in range(2)]
                for Hh in range(2):
                    pa.append(psp.tile([128, 512], f32, name=f"pa1{t}{Hh}", tag=f"ps{Hh}"))
                    gi_sb = gip.tile([128, 3, 256], f16, name=f"gi1{t}_{Hh}", tag="gi")
                    (nc.sync if Hh == 0 else nc.scalar).dma_start(
                        gi_sb[:],
                        ag1_out[jc][Hh*4:Hh*4+4, tl].rearrange("s b (c n) -> s b c n", c=3))
                    gis.append(gi_sb)
                def kgroup(kp, Hh):
                    for kk in range(8):
                        k = 8*kp + kk
                        lhsT = hT[kp][:, 32*kk:32*kk+32]
                        for j in range(4):
                            base = (Hh*4+j)*768
                            nc.tensor.matmul(out=pa[Hh][32*j:32*j+32, :], lhsT=lhsT,
                                rhs=whh[:, k, base:base+512],
                                start=(k == 0), stop=False, tile_position=(0, 32*j))
                        for j in range(4):
                            base = (Hh*4+j)*768
                            nc.tensor.matmul(out=pn[Hh][32*j:32*j+32, :], lhsT=lhsT,
                                rhs=whh[:, k, base+512:base+768],
                                start=(k == 0), stop=False, tile_position=(0, 32*j))
                kgroup(0, 0)
                kgroup(0, 1)
                kgroup(1, 0)
                for Hh in range(2):
                    if Hh == 1:
                        kgroup(1, 1)
                    gi_sb = gis[Hh]
                    nc.tensor.matmul(out=pa[Hh][:], lhsT=eye[:],
                                     rhs=gi_sb[:, 0:2, :].rearrange("p c n -> p (c n)"),
                                     start=False, stop=True)
                    nc.tensor.matmul(out=pn[Hh][:], lhsT=eye[:], rhs=bhn[:, Hh, :],
                                     start=False, stop=True)
                    rz = sb.tile([128, 512], f16, name=f"rz1{t}{Hh}", tag=f"rz{Hh}", bufs=1)
                    nc.scalar.activation(rz[:], pa[Hh][:], AF.Sigmoid, scale=1.0/S)
                    r = rz[:, 0:256]
                    z = rz[:, 256:512]
                    tn = sb.tile([128, 256], f16, name=f"tn1{t}{Hh}", tag="gt", bufs=3)
                    nc.vector.tensor_mul(tn[:], r, pn[Hh][:])
                    tn2 = sb.tile([128, 256], f16, name=f"tn21{t}{Hh}", tag="gt", bufs=3)
                    nc.vector.tensor_add(tn2[:], tn[:], gi_sb[:, 2, :])
                    n_ = sb.tile([128, 256], f16, name=f"n1{t}{Hh}", tag=f"n{Hh}", bufs=1)
                    nc.scalar.activation(n_[:], tn2[:], AF.Tanh, scale=1.0/S)
                    d_ = sb.tile([128, 256], f16, name=f"d1{t}{Hh}", tag="gt", bufs=3)
                    nc.vector.tensor_sub(d_[:], hg[Hh][:], n_[:])
                    zd = sb.tile([128, 256], f16, name=f"zd1{t}{Hh}", tag=f"zd{Hh}", bufs=1)
                    nc.vector.tensor_mul(zd[:], z, d_[:])
                    hn = hp.tile([128, 256], f16, name=f"hg{Hh}", tag=f"hg{Hh}")
                    nc.vector.tensor_add(hn[:], n_[:], zd[:])
                    newhg[Hh] = hn
                    nhT = hp.tile([128, 256], f16, name=f"hT{Hh}", tag=f"hT{Hh}")
                    nc.vector.transpose(nhT[:], hn[:])
                    nhT8 = hp.tile([128, 256], f8, name=f"hT8b{Hh}", tag=f"hT8b{Hh}")
                    nc.vector.tensor_copy(nhT8[:], nhT[:])
                    newhT[Hh] = nhT8
                    for j in range(4):
                        eng = nc.scalar if j % 2 == 0 else nc.sync
                        eng.dma_start(
                            out=hnat[t, :, Hh*1024:(Hh+1)*1024]
                                .rearrange("b (m j nl) -> b m j nl", m=8, j=4)[:, :, j, :],
                            in_=hn[32*j:32*j+32].rearrange("p (m nl) -> p m nl", m=8))
                hT = newhT
                hg = newhg

        # ================= run pipeline =================
        wz = sb.tile([1, 16], f16, name="wz", tag="wtile")
        nc.vector.memset(wz[:], 0.0)
        nc.sync.dma_start(out=warm_in[:, :], in_=wz[:])
        nc.gpsimd.collective_compute(
            "AllGather", mybir.AluOpType.bypass,
            replica_groups=[list(range(NC))],
            ins=[warm_in.ap().opt()], outs=[warm_out.ap().opt()])
        epoch0 = big_tile("epoch0", E0_PX + 2*20224 + 1536, dt=f8)
        phase_x0(epoch0[:, E0_PX:E0_PX + 2*20224].bitcast(f16))
        recurrence0(epoch0)
        recurrence1()

        # ================= downstream workspace =================
        ps_mod[0] = 4
        ds = big_tile("ds", 57344)  # (128, 57344) f16 = 112KB/p in the big slot
        def R(i, w=2048):
            return ds[:, 2048*i: 2048*i + w]

        # ---- emb gather ----
        selt = sb.tile([128, SH], f16, name="selt", tag="selt")
        nc.sync.dma_start(selt[:], sel_d[:, :])
        hflat = hnat.ap().rearrange("t b f -> t (b f)")
        eflat = emb_mine.ap().rearrange("s b f -> s (b f)")
        for ch in range(16):
            reg = R(2 * (ch % 2), 4096)
            eng_a = [nc.sync, nc.scalar, nc.gpsimd][ch % 3]
            eng_b = [nc.scalar, nc.gpsimd, nc.sync][ch % 3]
            eng_a.dma_start(reg[:, 0:2048], hflat[:, 4096*ch:4096*ch+2048])
            eng_b.dma_start(reg[:, 2048:4096], hflat[:, 4096*ch+2048:4096*ch+4096])
            emc = R(4 + 2 * (ch % 2), 4096)[0:SH, :]
            for q in range(8):
                pse = ps_tile([SH, 512], f"pse{ch}{q}")
                nc.tensor.matmul(out=pse[:], lhsT=selt[:],
                                 rhs=reg[:, 512*q:512*q+512], start=True, stop=True)
                if q % 2 == 0:
                    nc.vector.tensor_copy(emc[:, 512*q:512*q+512], pse[:])
                else:
                    nc.scalar.copy(emc[:, 512*q:512*q+512], pse[:])
            eng2 = nc.scalar if ch % 2 == 0 else nc.sync
            eng2.dma_start(out=eflat[:, 4096*ch:4096*ch+4096], in_=emc)
            if DEBUG:
                nc.scalar.dma_start(
                    out=dbg_emb.ap().rearrange("s b f -> s (b f)")[:, 4096*ch:4096*ch+4096],
                    in_=emc)

        # ---- attention ----
        awt = sb.tile([128, 32], f16, name="awt", tag="awt")
        nc.sync.dma_start(awt[:], awt_d[:, :])
        ab = sb.tile([128, 1], f32, name="ab", tag="ab")
        nc.sync.dma_start(ab[:], ab_d[:, :])
        od = sb.tile([128, 4], f16, name="od", tag="od")
        nc.sync.dma_start(od[:], od_d[:, :])
        od2 = sb.tile([4, 128], f16, name="od2", tag="od2")
        nc.sync.dma_start(od2[:], od2_d[:, :])
        vec16 = R(4)[0:16, :]            # (16, 2048) f16
        for g in range(4):
            Ast16 = R(5 + (g % 2))       # tanh(emb) f16 (128, 2048)
            Aload = R(7) if g % 2 == 0 else R(10)
            nc.sync.dma_start(Aload, emb_mine[4*g:4*g+4].rearrange("s b f -> (s b) f"))
            nc.scalar.activation(Ast16[:], Aload, AF.Tanh)
            EW = R(8 + (g % 2))
            for q in range(4):
                psaw = ps_tile([128, 512], f"psaw{g}{q}")
                for smp in range(4):
                    nc.tensor.matmul(
                        out=psaw[32*smp:32*smp+32, :],
                        lhsT=awt[32*smp:32*smp+32, :],
                        rhs=Ast16[32*smp:32*smp+32, 512*q:512*q+512],
                        start=True, stop=True, tile_position=(32*smp, 32*smp))
                nc.scalar.activation(EW[:, 512*q:512*q+512], psaw[:], AF.Exp,
                                     bias=ab[:, 0:1], scale=1.0)
                psd = ps_tile([4, 512], f"psd{g}{q}")
                nc.tensor.matmul(out=psd[:], lhsT=od[:], rhs=EW[:, 512*q:512*q+512],
                                 start=True, stop=True)
                Vu = R(12)[:, 512*q:512*q+512]
                nc.gpsimd.tensor_mul(Vu, EW[:, 512*q:512*q+512], Ast16[:, 512*q:512*q+512])
                psv = ps_tile([4, 512], f"psv{g}{q}")
                nc.tensor.matmul(out=psv[:], lhsT=od[:], rhs=Vu, start=True, stop=True)
                rden = R(11).bitcast(f32)[0:4, 512*(q % 2):512*(q % 2)+512]
                nc.vector.reciprocal_approx_fast(out=rden, in_=psd[:])
                vtmp = R(13)[0:4, 512*q:512*q+512]
                with nc.allow_low_precision(reason="softmax recip fp16 ok"):
                    nc.vector.tensor_mul(vtmp, psv[:], rden)
                nc.sync.dma_start(out=vec16[4*g:4*g+4, 512*q:512*q+512], in_=vtmp)
        attv = R(13)[0:16, :]
        nc.scalar.activation(attv, vec16, AF.Tanh)
        if DEBUG:
            nc.sync.dma_start(out=dbg_attv[:, :], in_=attv)

        # ---- build xnT (transposed features+ones) and xn_st ----
        zpad = R(26)[:, 256:320]
        nc.vector.memset(zpad, 0.0)
        nc.vector.memset(zpad[:, 0:1], 1.0)
        for gg in range(4):
            nc.sync.dma_start(out=att_pad[128*gg:128*gg+128, 64:128],
                              in_=zpad)
        for s in range(16):
            nc.scalar.dma_start(out=att_pad[32*s:32*s+32, 0:64],
                                in_=attv[s:s+1, :].rearrange("p (d h) -> p d h", d=32))
        xnT = R(14)[:, 0:512]
        nc.sync.dma_start_transpose(xnT, att_pad[:, :])
        xn_st = [R(14)[:, 512 + 64*g: 512 + 64*(g+1)] for g in range(4)]
        for g in range(4):
            for smp in range(4):
                nc.sync.dma_start(out=xn_st[g][32*smp:32*smp+32, :],
                                  in_=attv[4*g+smp:4*g+smp+1, :].rearrange("p (d h) -> p d h", d=32))

        # ---- GAT ----
        gatw = sb.tile([65, 4, 64], f16, name="gatw", tag="gatw")
        nc.sync.dma_start(gatw[:], gw_d[:, :, :])
        gatt = sb.tile([128, 2, 64], f16, name="gatt", tag="gatt")
        nc.sync.dma_start(gatt[:], gatt_d.ap().rearrange("l p h -> p l h"))
        gbias = sb.tile([128, 2, 64], f16, name="gbias", tag="gbias")
        nc.sync.dma_start(gbias[:], gbias_d.ap().rearrange("l p h -> p l h"))

        def gat_layer(L, xT_all, gout_off):
            """xT_all (128, 512) f16 [rows 0:65 = features+ones].
            writes tanh(gat(x)) to R(gout_off)[:, 64g:64g+64] per g."""
            for g in range(4):
                psx = ps_tile([128, 128], f"psx{L}{g}")
                for smp in range(4):
                    bs = 4*g + smp
                    for lr in range(2):
                        nc.tensor.matmul(out=psx[32*smp:32*smp+32, 64*lr:64*lr+64],
                                         lhsT=xT_all[0:65, 32*bs:32*bs+32],
                                         rhs=gatw[:, 2*L+lr, :], start=True, stop=True,
                                         tile_position=(0, 32*smp))
                xl = R(15)[:, 128*g:128*g+64]
                nc.vector.tensor_copy(xl, psx[:, 0:64])
                xr = R(15)[:, 128*g+64:128*g+128]
                nc.vector.tensor_copy(xr, psx[:, 64:128])
                xrf = (R(16) if g % 2 == 0 else R(7))[0:4, :]
                for smp in range(4):
                    nc.sync.dma_start(out=xrf[smp:smp+1, :].rearrange("p (d h) -> p d h", d=32),
                                      in_=xr[32*smp:32*smp+32, :])
                e3 = R(17 + g % 2)
                for q in range(4):
                    psxb = ps_tile([128, 512], f"psxb{L}{g}{q}")
                    nc.tensor.matmul(out=psxb[:], lhsT=od2[:], rhs=xrf[:, 512*q:512*q+512],
                                     start=True, stop=True)
                    e1 = R(19)[:, 1024*(g % 2):1024*(g % 2)+512]
                    nc.vector.tensor_add(
                        e1.rearrange("p (d h) -> p d h", d=8), psxb[:].rearrange("p (d h) -> p d h", d=8),
                        xl[:, None, :].broadcast_to([128, 8, 64]))
                    e2 = R(19)[:, 1024*(g % 2)+512:1024*(g % 2)+1024]
                    nc.scalar.activation(e2, e1, AF.Lrelu, alpha=0.2)
                    nc.vector.tensor_mul(
                        e3[:, 512*q:512*q+512].rearrange("p (d h) -> p d h", d=8),
                        e2.rearrange("p (d h) -> p d h", d=8),
                        gatt[:, L, :][:, None, :].broadcast_to([128, 8, 64]))
                RL = R(26) if g % 2 == 0 else R(13)
                lg = RL.bitcast(f32)[:, 208:240]
                nc.vector.tensor_reduce(lg, e3[:].rearrange("p (d h) -> p d h", d=32),
                                        axis=mybir.AxisListType.X, op=mybir.AluOpType.add)
                elg = RL[:, 320:352]
                nc.scalar.activation(elg, lg, AF.Exp)
                psd2 = ps_tile([4, 32], f"psd2{L}{g}")
                nc.tensor.matmul(out=psd2[:], lhsT=od[:], rhs=elg, start=True, stop=True)
                rd2f = RL.bitcast(f32)[0:4, 304:336]
                nc.vector.reciprocal_approx_fast(out=rd2f, in_=psd2[:])
                rd2 = RL[0:4, 384:416]
                nc.scalar.copy(rd2, rd2f)
                psb2 = ps_tile([128, 32], f"psb2{L}{g}")
                nc.tensor.matmul(out=psb2[:], lhsT=od2[:], rhs=rd2, start=True, stop=True)
                alp = RL[:, 352:384]
                nc.vector.tensor_mul(alp, elg, psb2[:])
                psg = ps_tile([128, 64], f"psg{L}{g}")
                for smp in range(4):
                    nc.tensor.matmul(out=psg[32*smp:32*smp+32, :],
                                     lhsT=alp[32*smp:32*smp+32, :],
                                     rhs=xl[32*smp:32*smp+32, :],
                                     start=True, stop=True,
                                     tile_position=(32*smp, 32*smp))
                gb = RL.bitcast(f32)[:, 240:304]
                nc.vector.tensor_add(gb, psg[:], gbias[:, L, :])
                nc.scalar.activation(R(gout_off)[:, 64*g:64*g+64], gb, AF.Tanh)

        gat_layer(0, xnT, 20)
        for gg in range(4):
            nc.sync.dma_start(out=att_pad[128*gg:128*gg+128, 64:128], in_=zpad)
            nc.scalar.dma_start(out=att_pad[128*gg:128*gg+128, 0:64],
                                in_=R(20)[:, 64*gg:64*gg+64])
        g0T = R(21)[:, 0:512]
        nc.sync.dma_start_transpose(g0T, att_pad[:, :])
        gat_layer(1, g0T, 22)
        if DEBUG:
            nc.sync.dma_start(out=dbg_g01[0], in_=R(20)[:, 0:64])
            nc.sync.dma_start(out=dbg_g01[1], in_=R(22)[:, 0:64])

        # ---- fusion ----
        for g in range(4):
            gs = R(21)[:, 512 + 64*g: 512 + 64*(g+1)]
            nc.vector.tensor_add(gs, R(20)[:, 64*g:64*g+64], R(22)[:, 64*g:64*g+64])
            nc.sync.dma_start(out=fus_nat[128*g:128*g+128, 0:64], in_=xn_st[g])
            nc.scalar.dma_start(out=fus_nat[128*g:128*g+128, 64:128], in_=gs)
        fusT = R(23)[:, 0:512]
        nc.sync.dma_start_transpose(fusT, fus_nat[:, :])

        # ---- caps ----
        fwt = sb.tile([65, 32], f16, name="fwt", tag="fwt")
        nc.sync.dma_start(fwt[:], fw_d[:, :])
        onesrow = R(9)[0:1, 0:512]
        nc.vector.memset(onesrow, 1.0)
        nc.sync.dma_start(out=caps_pad[64:65, :], in_=onesrow)
        RAs = [R(24), R(0), R(5), R(7)]
        RBs = [R(25), R(1), R(6), R(8)]
        for mt in range(16):
            RA = RAs[mt % 4]
            RB = RBs[mt % 4]
            wc = R(26)[:, 128*(mt % 4):128*(mt % 4)+128]
            nc.sync.dma_start(wc, wc_d[mt])
            pscap = ps_tile([128, 512], f"pscap{mt}")
            nc.tensor.matmul(out=pscap[:], lhsT=wc, rhs=fusT, start=True, stop=True)
            P = RA[:, 0:512]
            nc.vector.tensor_copy(P, pscap[:])
            o0 = sb.tile([128, 16], f32, name=f"o0{mt}", tag="o0", bufs=4)
            nc.vector.tensor_reduce(o0[:], P.rearrange("p (b c) -> p b c", b=16),
                                    axis=mybir.AxisListType.X, op=mybir.AluOpType.add)
            o0s = sb.tile([128, 16], f16, name=f"o0s{mt}", tag="o0s", bufs=4)
            nc.scalar.activation(o0s[:], o0[:], AF.Identity, scale=1.0/32.0)
            Lcur = RA[:, 512:1024]
            nc.vector.tensor_mul(Lcur.rearrange("p (b c) -> p b c", b=16),
                                 P.rearrange("p (b c) -> p b c", b=16),
                                 o0s[:][:, :, None].broadcast_to([128, 16, 32]))
            out_prev = o0s
            for it in (1, 2):
                Et = RA[:, 1024:1536]
                nc.scalar.activation(Et, Lcur, AF.Exp)
                EP = RA[:, 1536:2048]
                nc.gpsimd.tensor_mul(EP, Et, P)
                psdc = ps_tile([4, 512], f"psdc{mt}{it}")
                nc.tensor.matmul(out=psdc[:], lhsT=od[:], rhs=Et, start=True, stop=True)
                rdcf = (R(2 + mt % 2) if mt % 4 < 2 else R(9 + mt % 2)).bitcast(f32)[0:4, 512*(it-1):512*(it-1)+512]
                nc.vector.reciprocal_approx_fast(out=rdcf, in_=psdc[:])
                rdc = (R(27) if mt % 4 < 2 else R(11))[0:4, 512*(2*(mt % 2) + it - 1):512*(2*(mt % 2) + it - 1)+512]
                nc.gpsimd.tensor_copy(rdc, rdcf)
                psbc = ps_tile([128, 512], f"psbc{mt}{it}")
                nc.tensor.matmul(out=psbc[:], lhsT=od2[:], rhs=rdc, start=True, stop=True)
                pp = RB[:, 0:512]
                nc.vector.tensor_mul(pp, EP, psbc[:])
                oo = sb.tile([128, 16], f32, name=f"oo{mt}{it}", tag="o0", bufs=4)
                nc.vector.tensor_reduce(oo[:], pp.rearrange("p (b c) -> p b c", b=16),
                                        axis=mybir.AxisListType.X, op=mybir.AluOpType.add)
                oos = sb.tile([128, 16], f16, name=f"oos{mt}{it}", tag="oos", bufs=4)
                nc.gpsimd.tensor_copy(oos[:], oo[:])
                out_prev = oos
                if it == 1:
                    m2 = RB[:, 512:1024]
                    nc.vector.tensor_mul(m2.rearrange("p (b c) -> p b c", b=16),
                                         P.rearrange("p (b c) -> p b c", b=16),
                                         oos[:][:, :, None].broadcast_to([128, 16, 32]))
                    L2 = RB[:, 1024:1536]
                    nc.vector.tensor_add(L2, Lcur, m2)
                    Lcur = L2
            tc_t = sb.tile([128, 16], f16, name=f"tc{mt}", tag="tc", bufs=4)
            nc.scalar.activation(tc_t[:], out_prev[:], AF.Tanh)
            if DEBUG:
                nc.sync.dma_start(out=dbg_caps[mt], in_=tc_t[:])
            for l_loc in range(4):
                eng4 = nc.sync if l_loc % 2 == 0 else nc.gpsimd
                eng4.dma_start(
                    out=caps_pad[4*mt+l_loc].rearrange("(o s) -> o s", o=32),
                    in_=tc_t[32*l_loc:32*l_loc+32, :])
        capsT = R(23)[:, 512:1024]
        nc.sync.dma_start(capsT[0:65, :], caps_pad[0:65, :])
        psf = ps_tile([32, 512], "psf")
        nc.tensor.matmul(out=psf[:], lhsT=fwt[:], rhs=capsT[0:65, :], start=True, stop=True)
        fin = R(25)[0:32, 1024:1536]
        nc.scalar.activation(fin, psf[:], AF.Tanh)
        nc.sync.dma_start(out=out_d.ap().rearrange("dd o s -> dd (o s)"),
                          in_=fin)

        for p_ in reversed(ctxs):
            p_.__exit__(None, None, None)
    nc.compile()
    return nc


# ===================== host side =====================
_NC_CACHE = {}

def _get_program():
    if "prog" not in _NC_CACHE:
        _NC_CACHE["prog"] = build_program()
    return _NC_CACHE["prog"]


def _prep_inputs(inputs):
    X = np.asarray(inputs["inputs"], np.float32)
    X = np.nan_to_num(X, nan=0.0, posinf=1.0)
    ei = np.asarray(inputs["edge_index"])
    s = np.repeat(np.arange(D), D); t = np.tile(np.arange(D), D)
    off = (np.arange(B) * D)[:, None]
    exp_ei = np.stack([(s[None] + off).reshape(-1), (t[None] + off).reshape(-1)]).astype(ei.dtype)
    assert np.array_equal(ei, exp_ei), "edge_index mismatch vs block-diagonal pattern"

    # [B, T, F] -> [B, 128 part, KT*32] with partition-major contiguous layout
    xT = np.ascontiguousarray(
        np.swapaxes(X, 1, 2).reshape(B, KT, 128, 32).transpose(0, 2, 1, 3)
    ).reshape(B, 128, KT*32).astype(np.float16)

    wih0p = _gate_cols(np.asarray(inputs["Wih0"], np.float32)) * S
    wih1p = _gate_cols(np.asarray(inputs["Wih1"], np.float32)) * S
    whh0p = _gate_cols(np.asarray(inputs["Whh0"], np.float32)) * S
    whh1p = _gate_cols(np.asarray(inputs["Whh1"], np.float32)) * S
    whh0_dev = np.ascontiguousarray(whh0p.reshape(KT, 128, 6144)).astype(ml_dtypes.float8_e4m3)
    whh1_dev = np.ascontiguousarray(whh1p.reshape(KT, 128, 6144)).astype(ml_dtypes.float8_e4m3)

    def bias_strip(bih, bhh):
        b = np.zeros(6144, np.float32)
        for q in range(8):
            hds = PERM[q*256:(q+1)*256]
            b[q*768+0*256: q*768+1*256] = bih[0*2048 + hds] + bhh[0*2048 + hds]
            b[q*768+1*256: q*768+2*256] = bih[1*2048 + hds] + bhh[1*2048 + hds]
            b[q*768+2*256: q*768+3*256] = bih[2*2048 + hds]
        return b * S
    bih0 = np.asarray(inputs["bih0"], np.float32); bhh0 = np.asarray(inputs["bhh0"], np.float32)
    bih1 = np.asarray(inputs["bih1"], np.float32); bhh1 = np.asarray(inputs["bhh1"], np.float32)
    bs0 = bias_strip(bih0, bhh0).astype(np.float16)
    bs1 = bias_strip(bih1, bhh1).astype(np.float16)

    def bhn_bcast(bhh):
        outb = np.zeros((128, 2, 256), np.float32)
        for Hh in range(2):
            for j in range(4):
                hds = PERM[(Hh*4+j)*256:(Hh*4+j)*256+256]
                outb[32*j:32*j+32, Hh, :] = bhh[2*2048 + hds][None, :]
        return (outb * S).astype(np.float16)

    eye = np.eye(128, dtype=np.float16)
    A_w = np.asarray(inputs["A_w"], np.float32); A_b = np.asarray(inputs["A_b"], np.float32)
    awt = np.tile(A_w.T.astype(np.float16), (4, 1))
    ab = np.tile(A_b, 4)[:, None].astype(np.float32)
    od = np.zeros((128, 4), np.float16)
    for gq in range(4):
        od[32*gq:32*gq+32, gq] = 1.0
    od2 = np.ascontiguousarray(od.T)

    gw = np.zeros((65, 4, 64), np.float16)
    for L, pfx in enumerate(["g0", "g1"]):
        for lr, nm in enumerate(["l", "r"]):
            gw[0:64, 2*L+lr] = np.asarray(inputs[f"{pfx}_W{nm}"], np.float32).T.astype(np.float16)
            gw[64, 2*L+lr] = np.asarray(inputs[f"{pfx}_b{nm}"], np.float32).astype(np.float16)
    gat_att = np.zeros((2, 128, 64), np.float16)
    gat_bias = np.zeros((2, 128, 64), np.float16)
    for L, pfx in enumerate(["g0", "g1"]):
        gat_att[L] = np.tile(np.asarray(inputs[f"{pfx}_att"], np.float32), (128, 1)).astype(np.float16)
        gat_bias[L] = np.tile(np.asarray(inputs[f"{pfx}_bias"], np.float32), (128, 1)).astype(np.float16)

    Wc = np.asarray(inputs["W_caps"], np.float32)
    wc_t = np.zeros((16, 128, 128), np.float16)
    for mt in range(16):
        for l_loc in range(4):
            l = 4*mt + l_loc
            wc_t[mt, :, 32*l_loc:32*l_loc+32] = Wc[:, l, :].T.astype(np.float16)
    fw = np.zeros((65, 32), np.float16)
    fw[0:64] = np.asarray(inputs["F_w"], np.float32).T.astype(np.float16)
    fw[64] = np.asarray(inputs["F_b"], np.float32).astype(np.float16)

    common = dict(xT=xT, whh0=whh0_dev, whh1=whh1_dev,
                  bhn0=bhn_bcast(bhh0), bhn1=bhn_bcast(bhh1), eye=eye,
                  awt=awt, ab=ab, od=od, od2=od2, gw=gw, gat_att=gat_att,
                  gat_bias=gat_bias, wc=wc_t, fw=fw)
    in_maps = []
    for r in range(NC):
        sel = np.zeros((128, SH), np.float16)
        for i in range(SH):
            sel[SH*r + i, i] = 1.0
        m = dict(common)
        m["wih0"] = np.ascontiguousarray(wih0p[:, 768*r:768*r+768].astype(np.float16).reshape(KT, 128, 768))
        m["wih1"] = np.ascontiguousarray(
            wih1p[:, 768*r:768*r+768].astype(ml_dtypes.float8_e4m3).reshape(KT, 128, 768))
        m["b0"] = bs0[768*r:768*r+768][None, :].copy()
        m["b1"] = bs1[768*r:768*r+768][None, :].copy()
        m["sel"] = sel
        in_maps.append(m)
    return in_maps


def kernel(**inputs):
    in_maps = _prep_inputs(inputs)
    nc = _get_program()
    res = run_bass_kernel_spmd(nc, in_maps, list(range(NC)))
    out = np.concatenate([res.results[r]["out"].transpose(2, 1, 0) for r in range(NC)], axis=0)
    return out.astype(np.float32)


if __name__ == "__main__":
    t0 = time.time()
    build_program()
    print("build+compile", time.time() - t0)


# revision 22
# speedup vs baseline: 1.0094x; 1.0094x over previous
"""Trainium2 Bass kernel for nn_CapsGATattentionGRU (B=128, T=32, D=32, H=64, F=2048).

Sharding: GRU recurrence replicated on 8 cores (whh0 fp8 + whh1 fp16 SBUF-resident,
col-tiled packed matmuls, DVE block-transpose feedback); x-side gate inputs
N-sharded + AllGathered; layer-1 x-side matmuls merged into recurrence-0's
step boundaries (consuming SBUF-resident hT tiles); downstream batch-sharded
via one-hot gather matmul.
"""
import os, sys, time
sys.path.insert(0, '/opt/trn_rl_repo')
import numpy as np
import ml_dtypes

import concourse.bass as bass
import concourse.bacc as bacc
import concourse.tile as tile
from concourse import mybir
from concourse.bass_utils import run_bass_kernel_spmd

f8 = mybir.dt.float8e4
f16 = mybir.dt.float16
f32 = mybir.dt.float32
AF = mybir.ActivationFunctionType

D, T, H_, B = 32, 32, 64, 128
F = D * H_
KT = 16
NC = 8
SH = B // NC
S = 32.0   # fp8 weight prescale; ACT un-scale 1/S
DEBUG = os.environ.get("KBUILD_DEBUG", "") == "1"


def hd_perm():
    out = np.zeros(2048, np.int64)
    for Hh in range(2):
        for j in range(4):
            hds = Hh*1024 + (np.arange(8)[:, None]*128 + j*32 + np.arange(32)[None, :]).reshape(-1)
            out[(Hh*4+j)*256:(Hh*4+j)*256+256] = hds
    return out

PERM = hd_perm()


def _gate_cols(whmat):
    """whmat (6144, K) -> (K, 6144) transposed with perm'd col order."""
    K = whmat.shape[1]
    out = np.zeros((K, 6144), np.float32)
    for q in range(8):
        hds = PERM[q*256:(q+1)*256]
        for c in range(3):
            out[:, q*768 + c*256: q*768 + (c+1)*256] = whmat[c*2048 + hds].T
    return out


def build_program():
    nc = bacc.Bacc("TRN2", target_bir_lowering=False, debug=False, num_devices=NC)

    xT_d = nc.dram_tensor("xT", [B, 128, KT*32], f16, kind="ExternalInput")
    wih0_d = nc.dram_tensor("wih0", [KT, 128, 768], f16, kind="ExternalInput")
    wih1_d = nc.dram_tensor("wih1", [KT, 128, 768], f8, kind="ExternalInput")
    b0_d = nc.dram_tensor("b0", [1, 768], f16, kind="ExternalInput")
    b1_d = nc.dram_tensor("b1", [1, 768], f16, kind="ExternalInput")
    whh0_d = nc.dram_tensor("whh0", [KT, 128, 6144], f8, kind="ExternalInput")
    whh1_d = nc.dram_tensor("whh1", [KT, 128, 6144], f8, kind="ExternalInput")
    bhn0_d = nc.dram_tensor("bhn0", [128, 2, 256], f16, kind="ExternalInput")
    bhn1_d = nc.dram_tensor("bhn1", [128, 2, 256], f16, kind="ExternalInput")
    eye_d = nc.dram_tensor("eye", [128, 128], f16, kind="ExternalInput")
    sel_d = nc.dram_tensor("sel", [128, SH], f16, kind="ExternalInput")
    awt_d = nc.dram_tensor("awt", [128, 32], f16, kind="ExternalInput")
    ab_d = nc.dram_tensor("ab", [128, 1], f32, kind="ExternalInput")
    od_d = nc.dram_tensor("od", [128, 4], f16, kind="ExternalInput")
    od2_d = nc.dram_tensor("od2", [4, 128], f16, kind="ExternalInput")
    gw_d = nc.dram_tensor("gw", [65, 4, 64], f16, kind="ExternalInput")
    gatt_d = nc.dram_tensor("gat_att", [2, 128, 64], f16, kind="ExternalInput")
    gbias_d = nc.dram_tensor("gat_bias", [2, 128, 64], f16, kind="ExternalInput")
    wc_d = nc.dram_tensor("wc", [16, 128, 128], f16, kind="ExternalInput")
    fw_d = nc.dram_tensor("fw", [65, 32], f16, kind="ExternalInput")

    out_d = nc.dram_tensor("out", [32, 32, SH], f16, kind="ExternalOutput")
    if DEBUG:
        dbg_emb = nc.dram_tensor("dbg_emb", [SH, 32, 2048], f16, kind="ExternalOutput")
        dbg_attv = nc.dram_tensor("dbg_attv", [SH, 2048], f16, kind="ExternalOutput")
        dbg_g01 = nc.dram_tensor("dbg_g01", [2, 128, 64], f16, kind="ExternalOutput")
        dbg_caps = nc.dram_tensor("dbg_caps", [16, 128, 16], f16, kind="ExternalOutput")

    NCHUNK = 4
    CS = B // NCHUNK   # steps per AG chunk
    ag0_in = [nc.dram_tensor(f"ag0_in{j}", [CS, 32, 768], f16) for j in range(NCHUNK)]
    ag0_out = [nc.dram_tensor(f"ag0_out{j}", [NC, CS, 32, 768], f16, addr_space="Shared")
               for j in range(NCHUNK)]
    ag1_in = [nc.dram_tensor(f"ag1_in{j}", [CS, 32, 768], f16) for j in range(NCHUNK)]
    ag1_out = [nc.dram_tensor(f"ag1_out{j}", [NC, CS, 32, 768], f16, addr_space="Shared")
               for j in range(NCHUNK)]
    warm_in = nc.dram_tensor("warm_in", [1, 16], f16)
    warm_out = nc.dram_tensor("warm_out", [NC, 1, 16], f16, addr_space="Shared")
    hnat = nc.dram_tensor("hnat", [B, 32, 2048], f16)
    emb_mine = nc.dram_tensor("emb_mine", [SH, 32, 2048], f16)
    att_pad = nc.dram_tensor("att_pad", [512, 128], f16)
    fus_nat = nc.dram_tensor("fus_nat", [512, 128], f16)
    caps_pad = nc.dram_tensor("caps_pad", [128, 512], f16)

    with tile.TileContext(nc) as tc:
        ctxs = []
        def pool(**kw):
            p = tc.tile_pool(**kw)
            ctxs.append(p)
            return p.__enter__()
        wp = pool(name="wp", bufs=1)
        sb = pool(name="sb", bufs=1)
        gip = pool(name="gip", bufs=2)
        hp = pool(name="hp", bufs=2)
        psp = pool(name="ps", bufs=2, space="PSUM")

        # ---- psum tag rotation for downstream: 3 tags x 2 bufs (+px = 8 banks) ----
        ps_ctr = [0]
        ps_tags = ["ps0", "ps1", "ps2", "px"]
        ps_mod = [3]
        def ps_tile(shape, name):
            tag = ps_tags[ps_ctr[0] % ps_mod[0]]
            ps_ctr[0] += 1
            return psp.tile(shape, f32, name=name, tag=tag)

        # big-slot helper: one ~192KB slot (tag "big"), carved manually
        def big_tile(name, cols, dt=f16):
            return wp.tile([128, cols], dt, name=name, tag="big")

        # epoch0: phase_x0 f16 workspace + recurrence0 fp8 weights in ONE slot so
        # the whh0/wih1 loads overlap px0 compute.
        W0, W1 = KT*6144, KT*768
        E0_PX = W0 + W1 + 1536 + 2*768 + 5*512   # fp8-col offset of px0 region

        # ================= phase A: x-side of layer 0 =================
        def phase_x0(w):
            # layout (f16 cols): wih 12288 | xt 8x512 | bias 768 | bbb 768 | gio 6x384
            wih = w[:, 0:12288].rearrange("p (k n) -> p k n", k=KT)
            for k in range(KT):
                nc.sync.dma_start(wih[:, k, :], wih0_d[k])
            xts_ab = [[w[:, 12288+512*(4*ab_+s): 12288+512*(4*ab_+s+1)].rearrange("p (k b) -> p k b", k=KT)
                       for s in range(4)] for ab_ in range(2)]
            bb = w[0:1, 16384:17152]
            nc.sync.dma_start(bb, b0_d[:, :])
            bbb = w[:, 17152:17920]
            ones1 = sb.tile([1, 128], f16, name="ones1", tag="ones1")
            nc.vector.memset(ones1[:], 1.0)
            for half in range(2):
                pb = ps_tile([128, 384], f"pb{half}")
                nc.tensor.matmul(out=pb[:], lhsT=ones1[:], rhs=bb[:, 384*half:384*half+384],
                                 start=True, stop=True)
                nc.vector.tensor_copy(bbb[:, 384*half:384*half+384], pb[:])
            gio_off = 17920
            for p in range(B // 4):
                xts = xts_ab[p % 2]
                for s in range(4):
                    t = p*4 + s
                    eng = nc.sync if s % 2 == 0 else nc.scalar
                    eng.dma_start(xts[s][:], xT_d[t].rearrange("p (k b) -> p k b", k=KT))
                jc = (p * 4) // CS
                pl = p * 4 - jc * CS
                for ch in range(2):
                    ps = ps_tile([128, 384], f"psA{p}{ch}")
                    for k in range(KT):
                        for s in range(4):
                            nc.tensor.matmul(
                                out=ps[32*s:32*s+32, :],
                                lhsT=xts[s][:, k, :],
                                rhs=wih[:, k, 384*ch:384*ch+384],
                                start=(k == 0), stop=(k == KT-1),
                                tile_position=(0, 32*s))
                    gio = w[:, gio_off + ((p % 3)*2 + ch)*384: gio_off + ((p % 3)*2 + ch + 1)*384]
                    nc.vector.tensor_add(gio, ps[:], bbb[:, 384*ch:384*ch+384])
                    nc.scalar.dma_start(
                        out=ag0_in[jc][pl:pl+4, :, 384*ch:384*ch+384].rearrange("s b n -> (s b) n"),
                        in_=gio)
                if (p * 4) % CS == CS - 4:
                    nc.gpsimd.collective_compute(
                        "AllGather", mybir.AluOpType.bypass,
                        replica_groups=[list(range(NC))],
                        ins=[ag0_in[jc].ap().opt()], outs=[ag0_out[jc].ap().opt()])

        # ================= recurrence =================
        def recurrence0(w):
            """Layer-0 recurrence (fp8 whh) with layer-1 x-side (fp8 wih1)
            merged at 4-step boundaries; h fed back via DVE transpose + fp8 cast."""
            # carve (f8 cols): whh0 | wih1 | bbb1(f16) | gio1 x2 (f16) | hT8 ring 5x512
            whh = w[:, 0:W0].rearrange("p (k n) -> p k n", k=KT)
            for k in range(KT):
                eng = nc.sync if k % 2 == 0 else nc.scalar
                eng.dma_start(whh[:, k, :], whh0_d[k])
            wih1 = w[:, W0:W0+W1].rearrange("p (k n) -> p k n", k=KT)
            for k in range(KT):
                eng = nc.scalar if k % 2 == 0 else nc.sync
                eng.dma_start(wih1[:, k, :], wih1_d[k])
            bbb1 = w[:, W0+W1:W0+W1+1536].bitcast(f16)
            gio1 = [w[:, W0+W1+1536:W0+W1+2560].bitcast(f16),
                    w[:, W0+W1+2560:W0+W1+3072].bitcast(f16)]
            hT8ring = [w[:, W0+W1+3072+512*i:W0+W1+3072+512*(i+1)].rearrange(
                       "p (h n) -> p h n", h=2) for i in range(5)]
            # bbb1 broadcast via ones matmul
            bb1 = w[:, E0_PX + 2*20224: E0_PX + 2*20224 + 1536].bitcast(f16)[0:1, :]
            nc.sync.dma_start(bb1, b1_d[:, :])
            ones1 = sb.tile([1, 128], f16, name="ones1b", tag="ones1")
            nc.vector.memset(ones1[:], 1.0)
            for half in range(2):
                pb = psp.tile([128, 384], f32, name=f"pb1{half}", tag="px")
                nc.tensor.matmul(out=pb[:], lhsT=ones1[:], rhs=bb1[:, 384*half:384*half+384],
                                 start=True, stop=True)
                nc.vector.tensor_copy(bbb1[:, 384*half:384*half+384], pb[:])

            eye = sb.tile([128, 128], f16, name="eye0", tag="eye")
            nc.sync.dma_start(eye[:], eye_d[:, :])
            bhn = sb.tile([128, 2, 256], f16, name="bhn0", tag="bhn")
            nc.sync.dma_start(bhn[:], bhn0_d[:, :, :])
            wtile = sb.tile([1, 16], f16, name="wtile", tag="wtile")
            nc.scalar.dma_start(wtile[:], warm_out[0])
            nc.vector.tensor_add(bhn[0:1, 0, 0:16], bhn[0:1, 0, 0:16], wtile[:])
            hT = [hT8ring[0][:, Hh, :] for Hh in range(2)]
            hg = [hp.tile([128, 256], f16, name=f"hg{Hh}", tag=f"hg{Hh}", bufs=2) for Hh in range(2)]
            for Hh in range(2):
                nc.vector.memset(hT[Hh][:], 0.0)
                nc.vector.memset(hg[Hh][:], 0.0)
            hist = []

            def emit_px1(p):
                # layer-1 x-side for steps 4p..4p+3 from SBUF hT8 history
                jc = (p * 4) // CS
                pl = p * 4 - jc * CS
                for ch, (c0, cw) in enumerate(((0, 512), (512, 256))):
                    ps = psp.tile([128, cw], f32, name=f"px{p}{ch}", tag="px")
                    for k in range(KT):
                        kp, kk = k // 8, k % 8
                        for s in range(4):
                            nc.tensor.matmul(
                                out=ps[32*s:32*s+32, :],
                                lhsT=hist[4*p+s][kp][:, 32*kk:32*kk+32],
                                rhs=wih1[:, k, c0:c0+cw],
                                start=(k == 0), stop=(k == KT-1),
                                tile_position=(0, 32*s))
                    gio = gio1[ch][:, 0:cw]
                    nc.vector.tensor_add(gio, ps[:], bbb1[:, c0:c0+cw])
                    nc.scalar.dma_start(
                        out=ag1_in[jc][pl:pl+4, :, c0:c0+cw].rearrange("s b n -> (s b) n"),
                        in_=gio)
                if (p * 4) % CS == CS - 4:
                    nc.gpsimd.collective_compute(
                        "AllGather", mybir.AluOpType.bypass,
                        replica_groups=[list(range(NC))],
                        ins=[ag1_in[jc].ap().opt()], outs=[ag1_out[jc].ap().opt()])

            for t in range(B):
                jc, tl = t // CS, t % CS
                newhT = [None, None]
                newhg = [None, None]
                pa, gis = [], []
                pn = [psp.tile([128, 256], f32, name=f"pn{t}{Hh}", tag="ps2")
                      for Hh in range(2)]
                for Hh in range(2):
                    pa.append(psp.tile([128, 512], f32, name=f"pa{t}{Hh}", tag=f"ps{Hh}"))
                    gi_sb = gip.tile([128, 3, 256], f16, name=f"gi{t}_{Hh}", tag="gi")
                    nc.sync.dma_start(
                        gi_sb[:],
                        ag0_out[jc][Hh*4:Hh*4+4, tl].rearrange("s b (c n) -> s b c n", c=3))
                    gis.append(gi_sb)
                def kgroup(kp, Hh):
                    for kk in range(8):
                        k = 8*kp + kk
                        lhsT = hT[kp][:, 32*kk:32*kk+32]
                        for j in range(4):
                            base = (Hh*4+j)*768
                            nc.tensor.matmul(out=pa[Hh][32*j:32*j+32, :], lhsT=lhsT,
                                rhs=whh[:, k, base:base+512],
                                start=(k == 0), stop=False, tile_position=(0, 32*j))
                        for j in range(4):
                            base = (Hh*4+j)*768
                            nc.tensor.matmul(out=pn[Hh][32*j:32*j+32, :], lhsT=lhsT,
                                rhs=whh[:, k, base+512:base+768],
                                start=(k == 0), stop=False, tile_position=(0, 32*j))
                kgroup(0, 0)
                kgroup(0, 1)
                kgroup(1, 0)
                for Hh in range(2):
                    if Hh == 1:
                        kgroup(1, 1)
                    gi_sb = gis[Hh]
                    nc.tensor.matmul(out=pa[Hh][:], lhsT=eye[:],
                                     rhs=gi_sb[:, 0:2, :].rearrange("p c n -> p (c n)"),
                                     start=False, stop=True)
                    nc.tensor.matmul(out=pn[Hh][:], lhsT=eye[:], rhs=bhn[:, Hh, :],
                                     start=False, stop=True)
                    rz = sb.tile([128, 512], f16, name=f"rz{t}{Hh}", tag=f"rz{Hh}", bufs=1)
                    nc.scalar.activation(rz[:], pa[Hh][:], AF.Sigmoid, scale=1.0/S)
                    r = rz[:, 0:256]
                    z = rz[:, 256:512]
                    tn = sb.tile([128, 256], f16, name=f"tn{t}{Hh}", tag="gt", bufs=3)
                    nc.vector.tensor_mul(tn[:], r, pn[Hh][:])
                    tn2 = sb.tile([128, 256], f16, name=f"tn2{t}{Hh}", tag="gt", bufs=3)
                    nc.vector.tensor_add(tn2[:], tn[:], gi_sb[:, 2, :])
                    n_ = sb.tile([128, 256], f16, name=f"n{t}{Hh}", tag=f"n{Hh}", bufs=1)
                    nc.scalar.activation(n_[:], tn2[:], AF.Tanh, scale=1.0/S)
                    d_ = sb.tile([128, 256], f16, name=f"d{t}{Hh}", tag="gt", bufs=3)
                    nc.vector.tensor_sub(d_[:], hg[Hh][:], n_[:])
                    zd = sb.tile([128, 256], f16, name=f"zd{t}{Hh}", tag=f"zd{Hh}", bufs=1)
                    nc.vector.tensor_mul(zd[:], z, d_[:])
                    hn = hp.tile([128, 256], f16, name=f"hg{Hh}", tag=f"hg{Hh}")
                    nc.vector.tensor_add(hn[:], n_[:], zd[:])
                    newhg[Hh] = hn
                    nhT = hp.tile([128, 256], f16, name=f"hT{Hh}", tag=f"hT{Hh}", bufs=2)
                    nc.vector.transpose(nhT[:], hn[:])
                    nhT8 = hT8ring[(t + 1) % 5][:, Hh, :]
                    nc.vector.tensor_copy(nhT8, nhT[:])
                    newhT[Hh] = nhT8
                hT = newhT
                hg = newhg
                hist.append((newhT[0], newhT[1]))
                if t % 4 == 0 and t >= 4:
                    emit_px1(t // 4 - 1)
            emit_px1(31)

        def recurrence1():
            whh = wp.tile([128, KT, 6144], f8, name="whh1", tag="big")
            for k in range(KT):
                eng = nc.sync if k % 2 == 0 else nc.scalar
                eng.dma_start(whh[:, k, :], whh1_d[k])
            eye = sb.tile([128, 128], f16, name="eye1", tag="eye")
            nc.sync.dma_start(eye[:], eye_d[:, :])
            bhn = sb.tile([128, 2, 256], f16, name="bhn1", tag="bhn")
            nc.sync.dma_start(bhn[:], bhn1_d[:, :, :])
            hT = [hp.tile([128, 256], f8, name=f"hT8b{Hh}", tag=f"hT8b{Hh}", bufs=2)
                  for Hh in range(2)]
            hg = [hp.tile([128, 256], f16, name=f"hg{Hh}", tag=f"hg{Hh}", bufs=2) for Hh in range(2)]
            for Hh in range(2):
                nc.vector.memset(hT[Hh][:], 0.0)
                nc.vector.memset(hg[Hh][:], 0.0)
            for t in range(B):
                jc, tl = t // CS, t % CS
                newhT = [None, None]
                newhg = [None, None]
                pa, gis = [], []
                pn = [psp.tile([128, 256], f32, name=f"pn1{t}{Hh}", tag="ps2")
                      for Hh in range(2)]
                for Hh in range(2):
                    pa.append(psp.tile([128, 512], f32, name=f"pa1{t}{Hh}", tag=f"ps{Hh}"))
                    gi_sb = gip.tile([128, 3, 256], f16, name=f"gi1{t}_{Hh}", tag="gi")
                    (nc.sync if Hh == 0 else nc.scalar).dma_start(
                        gi_sb[:],
                        ag1_out[jc][Hh*4:Hh*4+4, tl].rearrange("s b (c n) -> s b c n", c=3))
                    gis.append(gi_sb)
                def kgroup(kp, Hh):
                    for kk in range(8):
                        k = 8*kp + kk
                        lhsT = hT[kp][:, 32*kk:32*kk+32]
                        for j in range(4):
                            base = (Hh*4+j)*768
                            nc.tensor.matmul(out=pa[Hh][32*j:32*j+32, :], lhsT=lhsT,
                                rhs=whh[:, k, base:base+512],
                                start=(k == 0), stop=False, tile_position=(0, 32*j))
                        for j in range(4):
                            base = (Hh*4+j)*768
                            nc.tensor.matmul(out=pn[Hh][32*j:32*j+32, :], lhsT=lhsT,
                                rhs=whh[:, k, base+512:base+768],
                                start=(k == 0), stop=False, tile_position=(0, 32*j))
                kgroup(0, 0)
                kgroup(0, 1)
                kgroup(1, 0)
                for Hh in range(2):
                    if Hh == 1:
                        kgroup(1, 1)
                    gi_sb = gis[Hh]
                    nc.tensor.matmul(out=pa[Hh][:], lhsT=eye[:],
                                     rhs=gi_sb[:, 0:2, :].rearrange("p c n -> p (c n)"),
                                     start=False, stop=True)
                    nc.tensor.matmul(out=pn[Hh][:], lhsT=eye[:], rhs=bhn[:, Hh, :],
                                     start=False, stop=True)
                    rz = sb.tile([128, 512], f16, name=f"rz1{t}{Hh}", tag=f"rz{Hh}", bufs=1)
                    nc.scalar.activation(rz[:], pa[Hh][:], AF.Sigmoid, scale=1.0/S)
                    r = rz[:, 0:256]
                    z = rz[:, 256:512]
                    tn = sb.tile([128, 256], f16, name=f"tn1{t}{Hh}", tag="gt", bufs=3)
                    nc.vector.tensor_mul(tn[:], r, pn[Hh][:])
                    tn2 = sb.tile([128, 256], f16, name=f"tn21{t}{Hh}", tag="gt", bufs=3)
                    nc.vector.tensor_add(tn2[:], tn[:], gi_sb[:, 2, :])
                    n_ = sb.tile([128, 256], f16, name=f"n1{t}{Hh}", tag=f"n{Hh}", bufs=1)
                    nc.scalar.activation(n_[:], tn2[:], AF.Tanh, scale=1.0/S)
                    d_ = sb.tile([128, 256], f16, name=f"d1{t}{Hh}", tag="gt", bufs=3)
                    nc.vector.tensor_sub(d_[:], hg[Hh][:], n_[:])
                    zd = sb.tile([128, 256], f16, name=f"zd1{t}{Hh}", tag=f"zd{Hh}", bufs=1)
                    nc.vector.tensor_mul(zd[:], z, d_[:])
                    hn = hp.tile([128, 256], f16, name=f"hg{Hh}", tag=f"hg{Hh}")
                    nc.vector.tensor_add(hn[:], n_[:], zd[:])
                    newhg[Hh] = hn
                    nhT = hp.tile([128, 256], f16, name=f"hT{Hh}", tag=f"hT{Hh}")
                    nc.vector.transpose(nhT[:], hn[:])
                    nhT8 = hp.tile([128, 256], f8, name=f"hT8b{Hh}", tag=f"hT8b{Hh}")
                    nc.vector.tensor_copy(nhT8[:], nhT[:])
                    newhT[Hh] = nhT8
                    for j in range(4):
                        eng = nc.scalar if j % 2 == 0 else nc.sync
                        eng.dma_start(
                            out=hnat[t, :, Hh*1024:(Hh+1)*1024]
                                .rearrange("b (m j nl) -> b m j nl", m=8, j=4)[:, :, j, :],
                            in_=hn[32*j:32*j+32].rearrange("p (m nl) -> p m nl", m=8))
                hT = newhT
                hg = newhg

        # ================= run pipeline =================
        wz = sb.tile([1, 16], f16, name="wz", tag="wtile")
        nc.vector.memset(wz[:], 0.0)
        nc.sync.dma_start(out=warm_in[:, :], in_=wz[:])
        nc.gpsimd.collective_compute(
            "AllGather", mybir.AluOpType.bypass,
            replica_groups=[list(range(NC))],
            ins=[warm_in.ap().opt()], outs=[warm_out.ap().opt()])
        epoch0 = big_tile("epoch0", E0_PX + 2*20224 + 1536, dt=f8)
        phase_x0(epoch0[:, E0_PX:E0_PX + 2*20224].bitcast(f16))
        recurrence0(epoch0)
        recurrence1()

        # ================= downstream workspace =================
        ps_mod[0] = 4
        ds = big_tile("ds", 57344)  # (128, 57344) f16 = 112KB/p in the big slot
        def R(i, w=2048):
            return ds[:, 2048*i: 2048*i + w]

        # ---- emb gather ----
        selt = sb.tile([128, SH], f16, name="selt", tag="selt")
        nc.sync.dma_start(selt[:], sel_d[:, :])
        hflat = hnat.ap().rearrange("t b f -> t (b f)")
        eflat = emb_mine.ap().rearrange("s b f -> s (b f)")
        for ch in range(16):
            reg = R(2 * (ch % 2), 4096)
            eng_a = [nc.sync, nc.scalar, nc.gpsimd][ch % 3]
            eng_b = [nc.scalar, nc.gpsimd, nc.sync][ch % 3]
            eng_a.dma_start(reg[:, 0:2048], hflat[:, 4096*ch:4096*ch+2048])
            eng_b.dma_start(reg[:, 2048:4096], hflat[:, 4096*ch+2048:4096*ch+4096])
            emc = R(4 + 2 * (ch % 2), 4096)[0:SH, :]
            for qq in range(2):
                pse = ps_tile([128, 512], f"pse{ch}{qq}")
                for q4 in range(4):
                    q = 4*qq + q4
                    nc.tensor.matmul(out=pse[32*q4:32*q4+SH, :], lhsT=selt[:],
                                     rhs=reg[:, 512*q:512*q+512], start=True, stop=True,
                                     tile_position=(0, 32*q4))
                for q4 in range(4):
                    q = 4*qq + q4
                    if q4 % 2 == 0:
                        nc.vector.tensor_copy(emc[:, 512*q:512*q+512], pse[32*q4:32*q4+SH, :])
                    else:
                        nc.scalar.copy(emc[:, 512*q:512*q+512], pse[32*q4:32*q4+SH, :])
            eng2 = nc.scalar if ch % 2 == 0 else nc.sync
            eng2.dma_start(out=eflat[:, 4096*ch:4096*ch+4096], in_=emc)
            if DEBUG:
                nc.scalar.dma_start(
                    out=dbg_emb.ap().rearrange("s b f -> s (b f)")[:, 4096*ch:4096*ch+4096],
                    in_=emc)

        # ---- attention ----
        awt = sb.tile([128, 32], f16, name="awt", tag="awt")
        nc.sync.dma_start(awt[:], awt_d[:, :])
        ab = sb.tile([128, 1], f32, name="ab", tag="ab")
        nc.sync.dma_start(ab[:], ab_d[:, :])
        od = sb.tile([128, 4], f16, name="od", tag="od")
        nc.sync.dma_start(od[:], od_d[:, :])
        od2 = sb.tile([4, 128], f16, name="od2", tag="od2")
        nc.sync.dma_start(od2[:], od2_d[:, :])
        vec16 = R(4)[0:16, :]            # (16, 2048) f16
        for g in range(4):
            Ast16 = R(5 + (g % 2))       # tanh(emb) f16 (128, 2048)
            Aload = R(7) if g % 2 == 0 else R(10)
            nc.sync.dma_start(Aload, emb_mine[4*g:4*g+4].rearrange("s b f -> (s b) f"))
            nc.scalar.activation(Ast16[:], Aload, AF.Tanh)
            EW = R(8 + (g % 2))
            for q in range(4):
                psaw = ps_tile([128, 512], f"psaw{g}{q}")
                for smp in range(4):
                    nc.tensor.matmul(
                        out=psaw[32*smp:32*smp+32, :],
                        lhsT=awt[32*smp:32*smp+32, :],
                        rhs=Ast16[32*smp:32*smp+32, 512*q:512*q+512],
                        start=True, stop=True, tile_position=(32*smp, 32*smp))
                nc.scalar.activation(EW[:, 512*q:512*q+512], psaw[:], AF.Exp,
                                     bias=ab[:, 0:1], scale=1.0)
                psd = ps_tile([4, 512], f"psd{g}{q}")
                nc.tensor.matmul(out=psd[:], lhsT=od[:], rhs=EW[:, 512*q:512*q+512],
                                 start=True, stop=True)
                Vu = R(12)[:, 512*q:512*q+512]
                nc.gpsimd.tensor_mul(Vu, EW[:, 512*q:512*q+512], Ast16[:, 512*q:512*q+512])
                psv = ps_tile([4, 512], f"psv{g}{q}")
                nc.tensor.matmul(out=psv[:], lhsT=od[:], rhs=Vu, start=True, stop=True)
                rden = R(11).bitcast(f32)[0:4, 512*(q % 2):512*(q % 2)+512]
                nc.vector.reciprocal_approx_fast(out=rden, in_=psd[:])
                vtmp = R(13)[0:4, 512*q:512*q+512]
                with nc.allow_low_precision(reason="softmax recip fp16 ok"):
                    nc.vector.tensor_mul(vtmp, psv[:], rden)
                nc.sync.dma_start(out=vec16[4*g:4*g+4, 512*q:512*q+512], in_=vtmp)
        attv = R(13)[0:16, :]
        nc.scalar.activation(attv, vec16, AF.Tanh)
        if DEBUG:
            nc.sync.dma_start(out=dbg_attv[:, :], in_=attv)

        # ---- build xnT (transposed features+ones) and xn_st ----
        zpad = R(26)[:, 256:320]
        nc.vector.memset(zpad, 0.0)
        nc.vector.memset(zpad[:, 0:1], 1.0)
        for gg in range(4):
            nc.sync.dma_start(out=att_pad[128*gg:128*gg+128, 64:128],
                              in_=zpad)
        for s in range(16):
            nc.scalar.dma_start(out=att_pad[32*s:32*s+32, 0:64],
                                in_=attv[s:s+1, :].rearrange("p (d h) -> p d h", d=32))
        xnT = R(14)[:, 0:512]
        nc.sync.dma_start_transpose(xnT, att_pad[:, :])
        xn_st = [R(14)[:, 512 + 64*g: 512 + 64*(g+1)] for g in range(4)]
        for g in range(4):
            for smp in range(4):
                nc.sync.dma_start(out=xn_st[g][32*smp:32*smp+32, :],
                                  in_=attv[4*g+smp:4*g+smp+1, :].rearrange("p (d h) -> p d h", d=32))

        # ---- GAT ----
        gatw = sb.tile([65, 4, 64], f16, name="gatw", tag="gatw")
        nc.sync.dma_start(gatw[:], gw_d[:, :, :])
        gatt = sb.tile([128, 2, 64], f16, name="gatt", tag="gatt")
        nc.sync.dma_start(gatt[:], gatt_d.ap().rearrange("l p h -> p l h"))
        gbias = sb.tile([128, 2, 64], f16, name="gbias", tag="gbias")
        nc.sync.dma_start(gbias[:], gbias_d.ap().rearrange("l p h -> p l h"))

        def gat_layer(L, xT_all, gout_off):
            """xT_all (128, 512) f16 [rows 0:65 = features+ones].
            writes tanh(gat(x)) to R(gout_off)[:, 64g:64g+64] per g."""
            for g in range(4):
                psx = ps_tile([128, 128], f"psx{L}{g}")
                for smp in range(4):
                    bs = 4*g + smp
                    for lr in range(2):
                        nc.tensor.matmul(out=psx[32*smp:32*smp+32, 64*lr:64*lr+64],
                                         lhsT=xT_all[0:65, 32*bs:32*bs+32],
                                         rhs=gatw[:, 2*L+lr, :], start=True, stop=True,
                                         tile_position=(0, 32*smp))
                xl = R(15)[:, 128*g:128*g+64]
                nc.vector.tensor_copy(xl, psx[:, 0:64])
                xr = R(15)[:, 128*g+64:128*g+128]
                nc.vector.tensor_copy(xr, psx[:, 64:128])
                xrf = (R(16) if g % 2 == 0 else R(7))[0:4, :]
                for smp in range(4):
                    nc.sync.dma_start(out=xrf[smp:smp+1, :].rearrange("p (d h) -> p d h", d=32),
                                      in_=xr[32*smp:32*smp+32, :])
                e3 = R(17 + g % 2)
                for q in range(4):
                    psxb = ps_tile([128, 512], f"psxb{L}{g}{q}")
                    nc.tensor.matmul(out=psxb[:], lhsT=od2[:], rhs=xrf[:, 512*q:512*q+512],
                                     start=True, stop=True)
                    e1 = R(19)[:, 1024*(g % 2):1024*(g % 2)+512]
                    nc.vector.tensor_add(
                        e1.rearrange("p (d h) -> p d h", d=8), psxb[:].rearrange("p (d h) -> p d h", d=8),
                        xl[:, None, :].broadcast_to([128, 8, 64]))
                    e2 = R(19)[:, 1024*(g % 2)+512:1024*(g % 2)+1024]
                    nc.scalar.activation(e2, e1, AF.Lrelu, alpha=0.2)
                    nc.vector.tensor_mul(
                        e3[:, 512*q:512*q+512].rearrange("p (d h) -> p d h", d=8),
                        e2.rearrange("p (d h) -> p d h", d=8),
                        gatt[:, L, :][:, None, :].broadcast_to([128, 8, 64]))
                RL = R(26) if g % 2 == 0 else R(13)
                lg = RL.bitcast(f32)[:, 208:240]
                nc.vector.tensor_reduce(lg, e3[:].rearrange("p (d h) -> p d h", d=32),
                                        axis=mybir.AxisListType.X, op=mybir.AluOpType.add)
                elg = RL[:, 320:352]
                nc.scalar.activation(elg, lg, AF.Exp)
                psd2 = ps_tile([4, 32], f"psd2{L}{g}")
                nc.tensor.matmul(out=psd2[:], lhsT=od[:], rhs=elg, start=True, stop=True)
                rd2f = RL.bitcast(f32)[0:4, 304:336]
                nc.vector.reciprocal_approx_fast(out=rd2f, in_=psd2[:])
                rd2 = RL[0:4, 384:416]
                nc.scalar.copy(rd2, rd2f)
                psb2 = ps_tile([128, 32], f"psb2{L}{g}")
                nc.tensor.matmul(out=psb2[:], lhsT=od2[:], rhs=rd2, start=True, stop=True)
                alp = RL[:, 352:384]
                nc.vector.tensor_mul(alp, elg, psb2[:])
                psg = ps_tile([128, 64], f"psg{L}{g}")
                for smp in range(4):
                    nc.tensor.matmul(out=psg[32*smp:32*smp+32, :],
                                     lhsT=alp[32*smp:32*smp+32, :],
                                     rhs=xl[32*smp:32*smp+32, :],
                                     start=True, stop=True,
                                     tile_position=(32*smp, 32*smp))
                gb = RL.bitcast(f32)[:, 240:304]
                nc.vector.tensor_add(gb, psg[:], gbias[:, L, :])
                nc.scalar.activation(R(gout_off)[:, 64*g:64*g+64], gb, AF.Tanh)

        gat_layer(0, xnT, 20)
        for gg in range(4):
            nc.sync.dma_start(out=att_pad[128*gg:128*gg+128, 64:128], in_=zpad)
            nc.scalar.dma_start(out=att_pad[128*gg:128*gg+128, 0:64],
                                in_=R(20)[:, 64*gg:64*gg+64])
        g0T = R(21)[:, 0:512]
        nc.sync.dma_start_transpose(g0T, att_pad[:, :])
        gat_layer(1, g0T, 22)
        if DEBUG:
            nc.sync.dma_start(out=dbg_g01[0], in_=R(20)[:, 0:64])
            nc.sync.dma_start(out=dbg_g01[1], in_=R(22)[:, 0:64])

        # ---- fusion ----
        for g in range(4):
            gs = R(21)[:, 512 + 64*g: 512 + 64*(g+1)]
            nc.vector.tensor_add(gs, R(20)[:, 64*g:64*g+64], R(22)[:, 64*g:64*g+64])
            nc.sync.dma_start(out=fus_nat[128*g:128*g+128, 0:64], in_=xn_st[g])
            nc.scalar.dma_start(out=fus_nat[128*g:128*g+128, 64:128], in_=gs)
        fusT = R(23)[:, 0:512]
        nc.sync.dma_start_transpose(fusT, fus_nat[:, :])

        # ---- caps ----
        fwt = sb.tile([65, 32], f16, name="fwt", tag="fwt")
        nc.sync.dma_start(fwt[:], fw_d[:, :])
        onesrow = R(9)[0:1, 0:512]
        nc.vector.memset(onesrow, 1.0)
        nc.sync.dma_start(out=caps_pad[64:65, :], in_=onesrow)
        RAs = [R(24), R(0), R(5), R(7)]
        RBs = [R(25), R(1), R(6), R(8)]
        for mt in range(16):
            RA = RAs[mt % 4]
            RB = RBs[mt % 4]
            wc = R(26)[:, 128*(mt % 4):128*(mt % 4)+128]
            nc.sync.dma_start(wc, wc_d[mt])
            pscap = ps_tile([128, 512], f"pscap{mt}")
            nc.tensor.matmul(out=pscap[:], lhsT=wc, rhs=fusT, start=True, stop=True)
            P = RA[:, 0:512]
            nc.vector.tensor_copy(P, pscap[:])
            o0 = sb.tile([128, 16], f32, name=f"o0{mt}", tag="o0", bufs=4)
            nc.vector.tensor_reduce(o0[:], P.rearrange("p (b c) -> p b c", b=16),
                                    axis=mybir.AxisListType.X, op=mybir.AluOpType.add)
            o0s = sb.tile([128, 16], f16, name=f"o0s{mt}", tag="o0s", bufs=4)
            nc.vector.tensor_scalar_mul(o0s[:], o0[:], 1.0/32.0)
            Lcur = RA[:, 512:1024]
            nc.vector.tensor_mul(Lcur.rearrange("p (b c) -> p b c", b=16),
                                 P.rearrange("p (b c) -> p b c", b=16),
                                 o0s[:][:, :, None].broadcast_to([128, 16, 32]))
            out_prev = o0s
            for it in (1, 2):
                Et = RA[:, 1024:1536]
                nc.scalar.activation(Et, Lcur, AF.Exp)
                EP = RA[:, 1536:2048]
                nc.gpsimd.tensor_mul(EP, Et, P)
                psdc = ps_tile([4, 512], f"psdc{mt}{it}")
                nc.tensor.matmul(out=psdc[:], lhsT=od[:], rhs=Et, start=True, stop=True)
                rdcf = (R(2 + mt % 2) if mt % 4 < 2 else R(9 + mt % 2)).bitcast(f32)[0:4, 512*(it-1):512*(it-1)+512]
                nc.vector.reciprocal_approx_fast(out=rdcf, in_=psdc[:])
                rdc = (R(27) if mt % 4 < 2 else R(11))[0:4, 512*(2*(mt % 2) + it - 1):512*(2*(mt % 2) + it - 1)+512]
                nc.gpsimd.tensor_copy(rdc, rdcf)
                psbc = ps_tile([128, 512], f"psbc{mt}{it}")
                nc.tensor.matmul(out=psbc[:], lhsT=od2[:], rhs=rdc, start=True, stop=True)
                pp = RB[:, 0:512]
                nc.vector.tensor_mul(pp, EP, psbc[:])
                oo = sb.tile([128, 16], f32, name=f"oo{mt}{it}", tag="o0", bufs=4)
                nc.vector.tensor_reduce(oo[:], pp.rearrange("p (b c) -> p b c", b=16),
                                        axis=mybir.AxisListType.X, op=mybir.AluOpType.add)
                oos = sb.tile([128, 16], f16, name=f"oos{mt}{it}", tag="oos", bufs=4)
                nc.gpsimd.tensor_copy(oos[:], oo[:])
                out_prev = oos
                if it == 1:
                    m2 = RB[:, 512:1024]
                    nc.vector.tensor_mul(m2.rearrange("p (b c) -> p b c", b=16),
                                         P.rearrange("p (b c) -> p b c", b=16),
                                         oos[:][:, :, None].broadcast_to([128, 16, 32]))
                    L2 = RB[:, 1024:1536]
                    nc.vector.tensor_add(L2, Lcur, m2)
                    Lcur = L2
            tc_t = sb.tile([128, 16], f16, name=f"tc{mt}", tag="tc", bufs=4)
            nc.scalar.activation(tc_t[:], out_prev[:], AF.Tanh)
            if DEBUG:
                nc.sync.dma_start(out=dbg_caps[mt], in_=tc_t[:])
            for l_loc in range(4):
                eng4 = nc.sync if l_loc % 2 == 0 else nc.gpsimd
                eng4.dma_start(
                    out=caps_pad[4*mt+l_loc].rearrange("(o s) -> o s", o=32),
                    in_=tc_t[32*l_loc:32*l_loc+32, :])
        capsT = R(23)[:, 512:1024]
        nc.sync.dma_start(capsT[0:65, :], caps_pad[0:65, :])
        psf = ps_tile([32, 512], "psf")
        nc.tensor.matmul(out=psf[:], lhsT=fwt[:], rhs=capsT[0:65, :], start=True, stop=True)
        fin = R(25)[0:32, 1024:1536]
        nc.scalar.activation(fin, psf[:], AF.Tanh)
        nc.sync.dma_start(out=out_d.ap().rearrange("dd o s -> dd (o s)"),
                          in_=fin)

        for p_ in reversed(ctxs):
            p_.__exit__(None, None, None)
    nc.compile()
    return nc


# ===================== host side =====================
_NC_CACHE = {}

def _get_program():
    if "prog" not in _NC_CACHE:
        _NC_CACHE["prog"] = build_program()
    return _NC_CACHE["prog"]


def _prep_inputs(inputs):
    X = np.asarray(inputs["inputs"], np.float32)
    X = np.nan_to_num(X, nan=0.0, posinf=1.0)
    ei = np.asarray(inputs["edge_index"])
    s = np.repeat(np.arange(D), D); t = np.tile(np.arange(D), D)
    off = (np.arange(B) * D)[:, None]
    exp_ei = np.stack([(s[None] + off).reshape(-1), (t[None] + off).reshape(-1)]).astype(ei.dtype)
    assert np.array_equal(ei, exp_ei), "edge_index mismatch vs block-diagonal pattern"

    # [B, T, F] -> [B, 128 part, KT*32] with partition-major contiguous layout
    xT = np.ascontiguousarray(
        np.swapaxes(X, 1, 2).reshape(B, KT, 128, 32).transpose(0, 2, 1, 3)
    ).reshape(B, 128, KT*32).astype(np.float16)

    wih0p = _gate_cols(np.asarray(inputs["Wih0"], np.float32)) * S
    wih1p = _gate_cols(np.asarray(inputs["Wih1"], np.float32)) * S
    whh0p = _gate_cols(np.asarray(inputs["Whh0"], np.float32)) * S
    whh1p = _gate_cols(np.asarray(inputs["Whh1"], np.float32)) * S
    whh0_dev = np.ascontiguousarray(whh0p.reshape(KT, 128, 6144)).astype(ml_dtypes.float8_e4m3)
    whh1_dev = np.ascontiguousarray(whh1p.reshape(KT, 128, 6144)).astype(ml_dtypes.float8_e4m3)

    def bias_strip(bih, bhh):
        b = np.zeros(6144, np.float32)
        for q in range(8):
            hds = PERM[q*256:(q+1)*256]
            b[q*768+0*256: q*768+1*256] = bih[0*2048 + hds] + bhh[0*2048 + hds]
            b[q*768+1*256: q*768+2*256] = bih[1*2048 + hds] + bhh[1*2048 + hds]
            b[q*768+2*256: q*768+3*256] = bih[2*2048 + hds]
        return b * S
    bih0 = np.asarray(inputs["bih0"], np.float32); bhh0 = np.asarray(inputs["bhh0"], np.float32)
    bih1 = np.asarray(inputs["bih1"], np.float32); bhh1 = np.asarray(inputs["bhh1"], np.float32)
    bs0 = bias_strip(bih0, bhh0).astype(np.float16)
    bs1 = bias_strip(bih1, bhh1).astype(np.float16)

    def bhn_bcast(bhh):
        outb = np.zeros((128, 2, 256), np.float32)
        for Hh in range(2):
            for j in range(4):
                hds = PERM[(Hh*4+j)*256:(Hh*4+j)*256+256]
                outb[32*j:32*j+32, Hh, :] = bhh[2*2048 + hds][None, :]
        return (outb * S).astype(np.float16)

    eye = np.eye(128, dtype=np.float16)
    A_w = np.asarray(inputs["A_w"], np.float32); A_b = np.asarray(inputs["A_b"], np.float32)
    awt = np.tile(A_w.T.astype(np.float16), (4, 1))
    ab = np.tile(A_b, 4)[:, None].astype(np.float32)
    od = np.zeros((128, 4), np.float16)
    for gq in range(4):
        od[32*gq:32*gq+32, gq] = 1.0
    od2 = np.ascontiguousarray(od.T)

    gw = np.zeros((65, 4, 64), np.float16)
    for L, pfx in enumerate(["g0", "g1"]):
        for lr, nm in enumerate(["l", "r"]):
            gw[0:64, 2*L+lr] = np.asarray(inputs[f"{pfx}_W{nm}"], np.float32).T.astype(np.float16)
            gw[64, 2*L+lr] = np.asarray(inputs[f"{pfx}_b{nm}"], np.float32).astype(np.float16)
    gat_att = np.zeros((2, 128, 64), np.float16)
    gat_bias = np.zeros((2, 128, 64), np.float16)
    for L, pfx in enumerate(["g0", "g1"]):
        gat_att[L] = np.tile(np.asarray(inputs[f"{pfx}_att"], np.float32), (128, 1)).astype(np.float16)
        gat_bias[L] = np.tile(np.asarray(inputs[f"{pfx}_bias"], np.float32), (128, 1)).astype(np.float16)

    Wc = np.asarray(inputs["W_caps"], np.float32)
    wc_t = np.zeros((16, 128, 128), np.float16)
    for mt in range(16):
        for l_loc in range(4):
            l = 4*mt + l_loc
            wc_t[mt, :, 32*l_loc:32*l_loc+32] = Wc[:, l, :].T.astype(np.float16)
    fw = np.zeros((65, 32), np.float16)
    fw[0:64] = np.asarray(inputs["F_w"], np.float32).T.astype(np.float16)
    fw[64] = np.asarray(inputs["F_b"], np.float32).astype(np.float16)

    common = dict(xT=xT, whh0=whh0_dev, whh1=whh1_dev,
                  bhn0=bhn_bcast(bhh0), bhn1=bhn_bcast(bhh1), eye=eye,
                  awt=awt, ab=ab, od=od, od2=od2, gw=gw, gat_att=gat_att,
                  gat_bias=gat_bias, wc=wc_t, fw=fw)
    in_maps = []
    for r in range(NC):
        sel = np.zeros((128, SH), np.float16)
        for i in range(SH):
            sel[SH*r + i, i] = 1.0
        m = dict(common)
        m["wih0"] = np.ascontiguousarray(wih0p[:, 768*r:768*r+768].astype(np.float16).reshape(KT, 128, 768))
        m["wih1"] = np.ascontiguousarray(
            wih1p[:, 768*r:768*r+768].astype(ml_dtypes.float8_e4m3).reshape(KT, 128, 768))
        m["b0"] = bs0[768*r:768*r+768][None, :].copy()
        m["b1"] = bs1[768*r:768*r+768][None, :].copy()
        m["sel"] = sel
        in_maps.append(m)
    return in_maps


def kernel(**inputs):
    in_maps = _prep_inputs(inputs)
    nc = _get_program()
    res = run_bass_kernel_spmd(nc, in_maps, list(range(NC)))
    out = np.concatenate([res.results[r]["out"].transpose(2, 1, 0) for r in range(NC)], axis=0)
    return out.astype(np.float32)


if __name__ == "__main__":
    t0 = time.time()
    build_program()
    print("build+compile", time.time() - t0)


# revision 23
# speedup vs baseline: 1.0129x; 1.0034x over previous
"""Trainium2 Bass kernel for nn_CapsGATattentionGRU (B=128, T=32, D=32, H=64, F=2048).

Sharding: GRU recurrence replicated on 8 cores (whh0 fp8 + whh1 fp16 SBUF-resident,
col-tiled packed matmuls, DVE block-transpose feedback); x-side gate inputs
N-sharded + AllGathered; layer-1 x-side matmuls merged into recurrence-0's
step boundaries (consuming SBUF-resident hT tiles); downstream batch-sharded
via one-hot gather matmul.
"""
import os, sys, time
sys.path.insert(0, '/opt/trn_rl_repo')
import numpy as np
import ml_dtypes

import concourse.bass as bass
import concourse.bacc as bacc
import concourse.tile as tile
from concourse import mybir
from concourse.bass_utils import run_bass_kernel_spmd

f8 = mybir.dt.float8e4
f16 = mybir.dt.float16
f32 = mybir.dt.float32
AF = mybir.ActivationFunctionType

D, T, H_, B = 32, 32, 64, 128
F = D * H_
KT = 16
NC = 8
SH = B // NC
S = 32.0   # fp8 weight prescale; ACT un-scale 1/S
DEBUG = os.environ.get("KBUILD_DEBUG", "") == "1"


def hd_perm():
    out = np.zeros(2048, np.int64)
    for Hh in range(2):
        for j in range(4):
            hds = Hh*1024 + (np.arange(8)[:, None]*128 + j*32 + np.arange(32)[None, :]).reshape(-1)
            out[(Hh*4+j)*256:(Hh*4+j)*256+256] = hds
    return out

PERM = hd_perm()


def _gate_cols(whmat):
    """whmat (6144, K) -> (K, 6144) transposed with perm'd col order."""
    K = whmat.shape[1]
    out = np.zeros((K, 6144), np.float32)
    for q in range(8):
        hds = PERM[q*256:(q+1)*256]
        for c in range(3):
            out[:, q*768 + c*256: q*768 + (c+1)*256] = whmat[c*2048 + hds].T
    return out


def build_program():
    nc = bacc.Bacc("TRN2", target_bir_lowering=False, debug=False, num_devices=NC)

    xT_d = nc.dram_tensor("xT", [B, 128, KT*32], f16, kind="ExternalInput")
    wih0_d = nc.dram_tensor("wih0", [KT, 128, 768], f16, kind="ExternalInput")
    wih1_d = nc.dram_tensor("wih1", [KT, 128, 768], f8, kind="ExternalInput")
    b0_d = nc.dram_tensor("b0", [1, 768], f16, kind="ExternalInput")
    b1_d = nc.dram_tensor("b1", [1, 768], f16, kind="ExternalInput")
    whh0_d = nc.dram_tensor("whh0", [KT, 128, 6144], f8, kind="ExternalInput")
    whh1_d = nc.dram_tensor("whh1", [KT, 128, 6144], f8, kind="ExternalInput")
    bhn0_d = nc.dram_tensor("bhn0", [128, 2, 256], f16, kind="ExternalInput")
    bhn1_d = nc.dram_tensor("bhn1", [128, 2, 256], f16, kind="ExternalInput")
    eye_d = nc.dram_tensor("eye", [128, 128], f16, kind="ExternalInput")
    sel_d = nc.dram_tensor("sel", [128, SH], f16, kind="ExternalInput")
    awt_d = nc.dram_tensor("awt", [128, 32], f16, kind="ExternalInput")
    ab_d = nc.dram_tensor("ab", [128, 1], f32, kind="ExternalInput")
    od_d = nc.dram_tensor("od", [128, 4], f16, kind="ExternalInput")
    od2_d = nc.dram_tensor("od2", [4, 128], f16, kind="ExternalInput")
    gw_d = nc.dram_tensor("gw", [65, 4, 64], f16, kind="ExternalInput")
    gatt_d = nc.dram_tensor("gat_att", [2, 128, 64], f16, kind="ExternalInput")
    gbias_d = nc.dram_tensor("gat_bias", [2, 128, 64], f16, kind="ExternalInput")
    wc_d = nc.dram_tensor("wc", [16, 128, 128], f16, kind="ExternalInput")
    fw_d = nc.dram_tensor("fw", [65, 32], f16, kind="ExternalInput")

    out_d = nc.dram_tensor("out", [32, 32, SH], f16, kind="ExternalOutput")
    if DEBUG:
        dbg_emb = nc.dram_tensor("dbg_emb", [SH, 32, 2048], f16, kind="ExternalOutput")
        dbg_attv = nc.dram_tensor("dbg_attv", [SH, 2048], f16, kind="ExternalOutput")
        dbg_g01 = nc.dram_tensor("dbg_g01", [2, 128, 64], f16, kind="ExternalOutput")
        dbg_caps = nc.dram_tensor("dbg_caps", [16, 128, 16], f16, kind="ExternalOutput")

    NCHUNK = 4
    CS = B // NCHUNK   # steps per AG chunk
    ag0_in = [nc.dram_tensor(f"ag0_in{j}", [CS, 32, 768], f16) for j in range(NCHUNK)]
    ag0_out = [nc.dram_tensor(f"ag0_out{j}", [NC, CS, 32, 768], f16, addr_space="Shared")
               for j in range(NCHUNK)]
    ag1_in = [nc.dram_tensor(f"ag1_in{j}", [CS, 32, 768], f16) for j in range(NCHUNK)]
    ag1_out = [nc.dram_tensor(f"ag1_out{j}", [NC, CS, 32, 768], f16, addr_space="Shared")
               for j in range(NCHUNK)]
    warm_in = nc.dram_tensor("warm_in", [1, 16], f16)
    warm_out = nc.dram_tensor("warm_out", [NC, 1, 16], f16, addr_space="Shared")
    hnat = nc.dram_tensor("hnat", [B, 32, 2048], f16)
    emb_mine = nc.dram_tensor("emb_mine", [SH, 32, 2048], f16)
    att_pad = nc.dram_tensor("att_pad", [512, 128], f16)
    fus_nat = nc.dram_tensor("fus_nat", [512, 128], f16)
    caps_pad = nc.dram_tensor("caps_pad", [128, 512], f16)

    with tile.TileContext(nc) as tc:
        ctxs = []
        def pool(**kw):
            p = tc.tile_pool(**kw)
            ctxs.append(p)
            return p.__enter__()
        wp = pool(name="wp", bufs=1)
        sb = pool(name="sb", bufs=1)
        gip = pool(name="gip", bufs=2)
        hp = pool(name="hp", bufs=2)
        psp = pool(name="ps", bufs=2, space="PSUM")

        # ---- psum tag rotation for downstream: 3 tags x 2 bufs (+px = 8 banks) ----
        ps_ctr = [0]
        ps_tags = ["ps0", "ps1", "ps2", "px"]
        ps_mod = [3]
        def ps_tile(shape, name):
            tag = ps_tags[ps_ctr[0] % ps_mod[0]]
            ps_ctr[0] += 1
            return psp.tile(shape, f32, name=name, tag=tag)

        # big-slot helper: one ~192KB slot (tag "big"), carved manually
        def big_tile(name, cols, dt=f16):
            return wp.tile([128, cols], dt, name=name, tag="big")

        # epoch0: phase_x0 f16 workspace + recurrence0 fp8 weights in ONE slot so
        # the whh0/wih1 loads overlap px0 compute.
        W0, W1 = KT*6144, KT*768
        E0_PX = W0 + W1 + 1536 + 2*768 + 5*512   # fp8-col offset of px0 region

        # ================= phase A: x-side of layer 0 =================
        def phase_x0(w):
            # layout (f16 cols): wih 12288 | xt 8x512 | bias 768 | bbb 768 | gio 6x384
            wih = w[:, 0:12288].rearrange("p (k n) -> p k n", k=KT)
            for k in range(KT):
                nc.sync.dma_start(wih[:, k, :], wih0_d[k])
            xts_ab = [[w[:, 12288+512*(4*ab_+s): 12288+512*(4*ab_+s+1)].rearrange("p (k b) -> p k b", k=KT)
                       for s in range(4)] for ab_ in range(2)]
            bb = w[0:1, 16384:17152]
            nc.sync.dma_start(bb, b0_d[:, :])
            bbb = w[:, 17152:17920]
            ones1 = sb.tile([1, 128], f16, name="ones1", tag="ones1")
            nc.vector.memset(ones1[:], 1.0)
            for half in range(2):
                pb = ps_tile([128, 384], f"pb{half}")
                nc.tensor.matmul(out=pb[:], lhsT=ones1[:], rhs=bb[:, 384*half:384*half+384],
                                 start=True, stop=True)
                nc.vector.tensor_copy(bbb[:, 384*half:384*half+384], pb[:])
            gio_off = 17920
            for p in range(B // 4):
                xts = xts_ab[p % 2]
                for s in range(4):
                    t = p*4 + s
                    eng = nc.sync if s % 2 == 0 else nc.scalar
                    eng.dma_start(xts[s][:], xT_d[t].rearrange("p (k b) -> p k b", k=KT))
                jc = (p * 4) // CS
                pl = p * 4 - jc * CS
                for ch in range(2):
                    ps = ps_tile([128, 384], f"psA{p}{ch}")
                    for k in range(KT):
                        for s in range(4):
                            nc.tensor.matmul(
                                out=ps[32*s:32*s+32, :],
                                lhsT=xts[s][:, k, :],
                                rhs=wih[:, k, 384*ch:384*ch+384],
                                start=(k == 0), stop=(k == KT-1),
                                tile_position=(0, 32*s))
                    gio = w[:, gio_off + ((p % 3)*2 + ch)*384: gio_off + ((p % 3)*2 + ch + 1)*384]
                    nc.vector.tensor_add(gio, ps[:], bbb[:, 384*ch:384*ch+384])
                    nc.scalar.dma_start(
                        out=ag0_in[jc][pl:pl+4, :, 384*ch:384*ch+384].rearrange("s b n -> (s b) n"),
                        in_=gio)
                if (p * 4) % CS == CS - 4:
                    nc.gpsimd.collective_compute(
                        "AllGather", mybir.AluOpType.bypass,
                        replica_groups=[list(range(NC))],
                        ins=[ag0_in[jc].ap().opt()], outs=[ag0_out[jc].ap().opt()])

        # ================= recurrence =================
        def recurrence0(w):
            """Layer-0 recurrence (fp8 whh) with layer-1 x-side (fp8 wih1)
            merged at 4-step boundaries; h fed back via DVE transpose + fp8 cast."""
            # carve (f8 cols): whh0 | wih1 | bbb1(f16) | gio1 x2 (f16) | hT8 ring 5x512
            whh = w[:, 0:W0].rearrange("p (k n) -> p k n", k=KT)
            for k in range(KT):
                eng = nc.sync if k % 2 == 0 else nc.scalar
                eng.dma_start(whh[:, k, :], whh0_d[k])
            wih1 = w[:, W0:W0+W1].rearrange("p (k n) -> p k n", k=KT)
            for k in range(KT):
                eng = nc.scalar if k % 2 == 0 else nc.sync
                eng.dma_start(wih1[:, k, :], wih1_d[k])
            bbb1 = w[:, W0+W1:W0+W1+1536].bitcast(f16)
            gio1 = [w[:, W0+W1+1536:W0+W1+2560].bitcast(f16),
                    w[:, W0+W1+2560:W0+W1+3072].bitcast(f16)]
            hT8ring = [w[:, W0+W1+3072+512*i:W0+W1+3072+512*(i+1)].rearrange(
                       "p (h n) -> p h n", h=2) for i in range(5)]
            # bbb1 broadcast via ones matmul
            bb1 = w[:, E0_PX + 2*20224: E0_PX + 2*20224 + 1536].bitcast(f16)[0:1, :]
            nc.sync.dma_start(bb1, b1_d[:, :])
            ones1 = sb.tile([1, 128], f16, name="ones1b", tag="ones1")
            nc.vector.memset(ones1[:], 1.0)
            for half in range(2):
                pb = psp.tile([128, 384], f32, name=f"pb1{half}", tag="px")
                nc.tensor.matmul(out=pb[:], lhsT=ones1[:], rhs=bb1[:, 384*half:384*half+384],
                                 start=True, stop=True)
                nc.vector.tensor_copy(bbb1[:, 384*half:384*half+384], pb[:])

            eye = sb.tile([128, 128], f16, name="eye0", tag="eye")
            nc.sync.dma_start(eye[:], eye_d[:, :])
            bhn = sb.tile([128, 2, 256], f16, name="bhn0", tag="bhn")
            nc.sync.dma_start(bhn[:], bhn0_d[:, :, :])
            wtile = sb.tile([1, 16], f16, name="wtile", tag="wtile")
            nc.scalar.dma_start(wtile[:], warm_out[0])
            nc.vector.tensor_add(bhn[0:1, 0, 0:16], bhn[0:1, 0, 0:16], wtile[:])
            hT = [hT8ring[0][:, Hh, :] for Hh in range(2)]
            hg = [hp.tile([128, 256], f16, name=f"hg{Hh}", tag=f"hg{Hh}", bufs=2) for Hh in range(2)]
            for Hh in range(2):
                nc.vector.memset(hT[Hh][:], 0.0)
                nc.vector.memset(hg[Hh][:], 0.0)
            hist = []

            def emit_px1(p):
                # layer-1 x-side for steps 4p..4p+3 from SBUF hT8 history
                jc = (p * 4) // CS
                pl = p * 4 - jc * CS
                for ch, (c0, cw) in enumerate(((0, 512), (512, 256))):
                    ps = psp.tile([128, cw], f32, name=f"px{p}{ch}", tag="px")
                    for k in range(KT):
                        kp, kk = k // 8, k % 8
                        for s in range(4):
                            nc.tensor.matmul(
                                out=ps[32*s:32*s+32, :],
                                lhsT=hist[4*p+s][kp][:, 32*kk:32*kk+32],
                                rhs=wih1[:, k, c0:c0+cw],
                                start=(k == 0), stop=(k == KT-1),
                                tile_position=(0, 32*s))
                    gio = gio1[ch][:, 0:cw]
                    nc.vector.tensor_add(gio, ps[:], bbb1[:, c0:c0+cw])
                    nc.scalar.dma_start(
                        out=ag1_in[jc][pl:pl+4, :, c0:c0+cw].rearrange("s b n -> (s b) n"),
                        in_=gio)
                if (p * 4) % CS == CS - 4:
                    nc.gpsimd.collective_compute(
                        "AllGather", mybir.AluOpType.bypass,
                        replica_groups=[list(range(NC))],
                        ins=[ag1_in[jc].ap().opt()], outs=[ag1_out[jc].ap().opt()])

            for t in range(B):
                jc, tl = t // CS, t % CS
                newhT = [None, None]
                newhg = [None, None]
                pa, gis = [], []
                pn = [psp.tile([128, 256], f32, name=f"pn{t}{Hh}", tag="ps2")
                      for Hh in range(2)]
                for Hh in range(2):
                    pa.append(psp.tile([128, 512], f32, name=f"pa{t}{Hh}", tag=f"ps{Hh}"))
                    gi_sb = gip.tile([128, 3, 256], f16, name=f"gi{t}_{Hh}", tag="gi", bufs=3)
                    nc.sync.dma_start(
                        gi_sb[:],
                        ag0_out[jc][Hh*4:Hh*4+4, tl].rearrange("s b (c n) -> s b c n", c=3))
                    gis.append(gi_sb)
                def kgroup(kp, Hh):
                    for kk in range(8):
                        k = 8*kp + kk
                        lhsT = hT[kp][:, 32*kk:32*kk+32]
                        for j in range(4):
                            base = (Hh*4+j)*768
                            nc.tensor.matmul(out=pa[Hh][32*j:32*j+32, :], lhsT=lhsT,
                                rhs=whh[:, k, base:base+512],
                                start=(k == 0), stop=False, tile_position=(0, 32*j))
                        for j in range(4):
                            base = (Hh*4+j)*768
                            nc.tensor.matmul(out=pn[Hh][32*j:32*j+32, :], lhsT=lhsT,
                                rhs=whh[:, k, base+512:base+768],
                                start=(k == 0), stop=False, tile_position=(0, 32*j))
                kgroup(0, 0)
                kgroup(0, 1)
                kgroup(1, 0)
                for Hh in range(2):
                    if Hh == 1:
                        kgroup(1, 1)
                    gi_sb = gis[Hh]
                    nc.tensor.matmul(out=pa[Hh][:], lhsT=eye[:],
                                     rhs=gi_sb[:, 0:2, :].rearrange("p c n -> p (c n)"),
                                     start=False, stop=True)
                    nc.tensor.matmul(out=pn[Hh][:], lhsT=eye[:], rhs=bhn[:, Hh, :],
                                     start=False, stop=True)
                    rz = sb.tile([128, 512], f16, name=f"rz{t}{Hh}", tag=f"rz{Hh}", bufs=2)
                    nc.scalar.activation(rz[:], pa[Hh][:], AF.Sigmoid, scale=1.0/S)
                    r = rz[:, 0:256]
                    z = rz[:, 256:512]
                    tn = sb.tile([128, 256], f16, name=f"tn{t}{Hh}", tag="gt", bufs=3)
                    nc.vector.tensor_mul(tn[:], r, pn[Hh][:])
                    tn2 = sb.tile([128, 256], f16, name=f"tn2{t}{Hh}", tag="gt", bufs=3)
                    nc.vector.tensor_add(tn2[:], tn[:], gi_sb[:, 2, :])
                    n_ = sb.tile([128, 256], f16, name=f"n{t}{Hh}", tag=f"n{Hh}", bufs=2)
                    nc.scalar.activation(n_[:], tn2[:], AF.Tanh, scale=1.0/S)
                    d_ = sb.tile([128, 256], f16, name=f"d{t}{Hh}", tag="gt", bufs=3)
                    nc.vector.tensor_sub(d_[:], hg[Hh][:], n_[:])
                    zd = sb.tile([128, 256], f16, name=f"zd{t}{Hh}", tag=f"zd{Hh}", bufs=2)
                    nc.vector.tensor_mul(zd[:], z, d_[:])
                    hn = hp.tile([128, 256], f16, name=f"hg{Hh}", tag=f"hg{Hh}")
                    nc.vector.tensor_add(hn[:], n_[:], zd[:])
                    newhg[Hh] = hn
                    nhT = hp.tile([128, 256], f16, name=f"hT{Hh}", tag=f"hT{Hh}", bufs=2)
                    nc.vector.transpose(nhT[:], hn[:])
                    nhT8 = hT8ring[(t + 1) % 5][:, Hh, :]
                    nc.vector.tensor_copy(nhT8, nhT[:])
                    newhT[Hh] = nhT8
                hT = newhT
                hg = newhg
                hist.append((newhT[0], newhT[1]))
                if t % 4 == 0 and t >= 4:
                    emit_px1(t // 4 - 1)
            emit_px1(31)

        def recurrence1():
            whh = wp.tile([128, KT, 6144], f8, name="whh1", tag="big")
            for k in range(KT):
                eng = nc.sync if k % 2 == 0 else nc.scalar
                eng.dma_start(whh[:, k, :], whh1_d[k])
            eye = sb.tile([128, 128], f16, name="eye1", tag="eye")
            nc.sync.dma_start(eye[:], eye_d[:, :])
            bhn = sb.tile([128, 2, 256], f16, name="bhn1", tag="bhn")
            nc.sync.dma_start(bhn[:], bhn1_d[:, :, :])
            hT = [hp.tile([128, 256], f8, name=f"hT8b{Hh}", tag=f"hT8b{Hh}", bufs=2)
                  for Hh in range(2)]
            hg = [hp.tile([128, 256], f16, name=f"hg{Hh}", tag=f"hg{Hh}", bufs=2) for Hh in range(2)]
            for Hh in range(2):
                nc.vector.memset(hT[Hh][:], 0.0)
                nc.vector.memset(hg[Hh][:], 0.0)
            for t in range(B):
                jc, tl = t // CS, t % CS
                newhT = [None, None]
                newhg = [None, None]
                pa, gis = [], []
                pn = [psp.tile([128, 256], f32, name=f"pn1{t}{Hh}", tag="ps2")
                      for Hh in range(2)]
                for Hh in range(2):
                    pa.append(psp.tile([128, 512], f32, name=f"pa1{t}{Hh}", tag=f"ps{Hh}"))
                    gi_sb = gip.tile([128, 3, 256], f16, name=f"gi1{t}_{Hh}", tag="gi", bufs=3)
                    (nc.sync if Hh == 0 else nc.scalar).dma_start(
                        gi_sb[:],
                        ag1_out[jc][Hh*4:Hh*4+4, tl].rearrange("s b (c n) -> s b c n", c=3))
                    gis.append(gi_sb)
                def kgroup(kp, Hh):
                    for kk in range(8):
                        k = 8*kp + kk
                        lhsT = hT[kp][:, 32*kk:32*kk+32]
                        for j in range(4):
                            base = (Hh*4+j)*768
                            nc.tensor.matmul(out=pa[Hh][32*j:32*j+32, :], lhsT=lhsT,
                                rhs=whh[:, k, base:base+512],
                                start=(k == 0), stop=False, tile_position=(0, 32*j))
                        for j in range(4):
                            base = (Hh*4+j)*768
                            nc.tensor.matmul(out=pn[Hh][32*j:32*j+32, :], lhsT=lhsT,
                                rhs=whh[:, k, base+512:base+768],
                                start=(k == 0), stop=False, tile_position=(0, 32*j))
                kgroup(0, 0)
                kgroup(0, 1)
                kgroup(1, 0)
                for Hh in range(2):
                    if Hh == 1:
                        kgroup(1, 1)
                    gi_sb = gis[Hh]
                    nc.tensor.matmul(out=pa[Hh][:], lhsT=eye[:],
                                     rhs=gi_sb[:, 0:2, :].rearrange("p c n -> p (c n)"),
                                     start=False, stop=True)
                    nc.tensor.matmul(out=pn[Hh][:], lhsT=eye[:], rhs=bhn[:, Hh, :],
                                     start=False, stop=True)
                    rz = sb.tile([128, 512], f16, name=f"rz1{t}{Hh}", tag=f"rz{Hh}", bufs=2)
                    nc.scalar.activation(rz[:], pa[Hh][:], AF.Sigmoid, scale=1.0/S)
                    r = rz[:, 0:256]
                    z = rz[:, 256:512]
                    tn = sb.tile([128, 256], f16, name=f"tn1{t}{Hh}", tag="gt", bufs=3)
                    nc.vector.tensor_mul(tn[:], r, pn[Hh][:])
                    tn2 = sb.tile([128, 256], f16, name=f"tn21{t}{Hh}", tag="gt", bufs=3)
                    nc.vector.tensor_add(tn2[:], tn[:], gi_sb[:, 2, :])
                    n_ = sb.tile([128, 256], f16, name=f"n1{t}{Hh}", tag=f"n{Hh}", bufs=2)
                    nc.scalar.activation(n_[:], tn2[:], AF.Tanh, scale=1.0/S)
                    d_ = sb.tile([128, 256], f16, name=f"d1{t}{Hh}", tag="gt", bufs=3)
                    nc.vector.tensor_sub(d_[:], hg[Hh][:], n_[:])
                    zd = sb.tile([128, 256], f16, name=f"zd1{t}{Hh}", tag=f"zd{Hh}", bufs=2)
                    nc.vector.tensor_mul(zd[:], z, d_[:])
                    hn = hp.tile([128, 256], f16, name=f"hg{Hh}", tag=f"hg{Hh}")
                    nc.vector.tensor_add(hn[:], n_[:], zd[:])
                    newhg[Hh] = hn
                    nhT = hp.tile([128, 256], f16, name=f"hT{Hh}", tag=f"hT{Hh}")
                    nc.vector.transpose(nhT[:], hn[:])
                    nhT8 = hp.tile([128, 256], f8, name=f"hT8b{Hh}", tag=f"hT8b{Hh}")
                    nc.vector.tensor_copy(nhT8[:], nhT[:])
                    newhT[Hh] = nhT8
                    for j in range(4):
                        eng = nc.scalar if j % 2 == 0 else nc.sync
                        eng.dma_start(
                            out=hnat[t, :, Hh*1024:(Hh+1)*1024]
                                .rearrange("b (m j nl) -> b m j nl", m=8, j=4)[:, :, j, :],
                            in_=hn[32*j:32*j+32].rearrange("p (m nl) -> p m nl", m=8))
                hT = newhT
                hg = newhg

        # ================= run pipeline =================
        wz = sb.tile([1, 16], f16, name="wz", tag="wtile")
        nc.vector.memset(wz[:], 0.0)
        nc.sync.dma_start(out=warm_in[:, :], in_=wz[:])
        nc.gpsimd.collective_compute(
            "AllGather", mybir.AluOpType.bypass,
            replica_groups=[list(range(NC))],
            ins=[warm_in.ap().opt()], outs=[warm_out.ap().opt()])
        epoch0 = big_tile("epoch0", E0_PX + 2*20224 + 1536, dt=f8)
        phase_x0(epoch0[:, E0_PX:E0_PX + 2*20224].bitcast(f16))
        recurrence0(epoch0)
        recurrence1()

        # ================= downstream workspace =================
        ps_mod[0] = 4
        ds = big_tile("ds", 57344)  # (128, 57344) f16 = 112KB/p in the big slot
        def R(i, w=2048):
            return ds[:, 2048*i: 2048*i + w]

        # ---- emb gather ----
        selt = sb.tile([128, SH], f16, name="selt", tag="selt")
        nc.sync.dma_start(selt[:], sel_d[:, :])
        hflat = hnat.ap().rearrange("t b f -> t (b f)")
        eflat = emb_mine.ap().rearrange("s b f -> s (b f)")
        for ch in range(16):
            reg = R(2 * (ch % 2), 4096)
            eng_a = [nc.sync, nc.scalar, nc.gpsimd][ch % 3]
            eng_b = [nc.scalar, nc.gpsimd, nc.sync][ch % 3]
            eng_a.dma_start(reg[:, 0:2048], hflat[:, 4096*ch:4096*ch+2048])
            eng_b.dma_start(reg[:, 2048:4096], hflat[:, 4096*ch+2048:4096*ch+4096])
            emc = R(4 + 2 * (ch % 2), 4096)[0:SH, :]
            for qq in range(2):
                pse = ps_tile([128, 512], f"pse{ch}{qq}")
                for q4 in range(4):
                    q = 4*qq + q4
                    nc.tensor.matmul(out=pse[32*q4:32*q4+SH, :], lhsT=selt[:],
                                     rhs=reg[:, 512*q:512*q+512], start=True, stop=True,
                                     tile_position=(0, 32*q4))
                for q4 in range(4):
                    q = 4*qq + q4
                    if q4 % 2 == 0:
                        nc.vector.tensor_copy(emc[:, 512*q:512*q+512], pse[32*q4:32*q4+SH, :])
                    else:
                        nc.scalar.copy(emc[:, 512*q:512*q+512], pse[32*q4:32*q4+SH, :])
            eng2 = nc.scalar if ch % 2 == 0 else nc.sync
            eng2.dma_start(out=eflat[:, 4096*ch:4096*ch+4096], in_=emc)
            if DEBUG:
                nc.scalar.dma_start(
                    out=dbg_emb.ap().rearrange("s b f -> s (b f)")[:, 4096*ch:4096*ch+4096],
                    in_=emc)

        # ---- attention ----
        awt = sb.tile([128, 32], f16, name="awt", tag="awt")
        nc.sync.dma_start(awt[:], awt_d[:, :])
        ab = sb.tile([128, 1], f32, name="ab", tag="ab")
        nc.sync.dma_start(ab[:], ab_d[:, :])
        od = sb.tile([128, 4], f16, name="od", tag="od")
        nc.sync.dma_start(od[:], od_d[:, :])
        od2 = sb.tile([4, 128], f16, name="od2", tag="od2")
        nc.sync.dma_start(od2[:], od2_d[:, :])
        vec16 = R(4)[0:16, :]            # (16, 2048) f16
        for g in range(4):
            Ast16 = R(5 + (g % 2))       # tanh(emb) f16 (128, 2048)
            Aload = R(7) if g % 2 == 0 else R(10)
            nc.sync.dma_start(Aload, emb_mine[4*g:4*g+4].rearrange("s b f -> (s b) f"))
            nc.scalar.activation(Ast16[:], Aload, AF.Tanh)
            EW = R(8 + (g % 2))
            for q in range(4):
                psaw = ps_tile([128, 512], f"psaw{g}{q}")
                for smp in range(4):
                    nc.tensor.matmul(
                        out=psaw[32*smp:32*smp+32, :],
                        lhsT=awt[32*smp:32*smp+32, :],
                        rhs=Ast16[32*smp:32*smp+32, 512*q:512*q+512],
                        start=True, stop=True, tile_position=(32*smp, 32*smp))
                nc.scalar.activation(EW[:, 512*q:512*q+512], psaw[:], AF.Exp,
                                     bias=ab[:, 0:1], scale=1.0)
                psd = ps_tile([4, 512], f"psd{g}{q}")
                nc.tensor.matmul(out=psd[:], lhsT=od[:], rhs=EW[:, 512*q:512*q+512],
                                 start=True, stop=True)
                Vu = R(12)[:, 512*q:512*q+512]
                nc.gpsimd.tensor_mul(Vu, EW[:, 512*q:512*q+512], Ast16[:, 512*q:512*q+512])
                psv = ps_tile([4, 512], f"psv{g}{q}")
                nc.tensor.matmul(out=psv[:], lhsT=od[:], rhs=Vu, start=True, stop=True)
                rden = R(11).bitcast(f32)[0:4, 512*(q % 2):512*(q % 2)+512]
                nc.vector.reciprocal_approx_fast(out=rden, in_=psd[:])
                vtmp = R(13)[0:4, 512*q:512*q+512]
                with nc.allow_low_precision(reason="softmax recip fp16 ok"):
                    nc.vector.tensor_mul(vtmp, psv[:], rden)
                nc.sync.dma_start(out=vec16[4*g:4*g+4, 512*q:512*q+512], in_=vtmp)
        attv = R(13)[0:16, :]
        nc.scalar.activation(attv, vec16, AF.Tanh)
        if DEBUG:
            nc.sync.dma_start(out=dbg_attv[:, :], in_=attv)

        # ---- build xnT (transposed features+ones) and xn_st ----
        zpad = R(26)[:, 256:320]
        nc.vector.memset(zpad, 0.0)
        nc.vector.memset(zpad[:, 0:1], 1.0)
        for gg in range(4):
            nc.sync.dma_start(out=att_pad[128*gg:128*gg+128, 64:128],
                              in_=zpad)
        for s in range(16):
            nc.scalar.dma_start(out=att_pad[32*s:32*s+32, 0:64],
                                in_=attv[s:s+1, :].rearrange("p (d h) -> p d h", d=32))
        xnT = R(14)[:, 0:512]
        nc.sync.dma_start_transpose(xnT, att_pad[:, :])
        xn_st = [R(14)[:, 512 + 64*g: 512 + 64*(g+1)] for g in range(4)]
        for g in range(4):
            for smp in range(4):
                nc.sync.dma_start(out=xn_st[g][32*smp:32*smp+32, :],
                                  in_=attv[4*g+smp:4*g+smp+1, :].rearrange("p (d h) -> p d h", d=32))

        # ---- GAT ----
        gatw = sb.tile([65, 4, 64], f16, name="gatw", tag="gatw")
        nc.sync.dma_start(gatw[:], gw_d[:, :, :])
        gatt = sb.tile([128, 2, 64], f16, name="gatt", tag="gatt")
        nc.sync.dma_start(gatt[:], gatt_d.ap().rearrange("l p h -> p l h"))
        gbias = sb.tile([128, 2, 64], f16, name="gbias", tag="gbias")
        nc.sync.dma_start(gbias[:], gbias_d.ap().rearrange("l p h -> p l h"))

        def gat_layer(L, xT_all, gout_off):
            """xT_all (128, 512) f16 [rows 0:65 = features+ones].
            writes tanh(gat(x)) to R(gout_off)[:, 64g:64g+64] per g."""
            for g in range(4):
                psx = ps_tile([128, 128], f"psx{L}{g}")
                for smp in range(4):
                    bs = 4*g + smp
                    for lr in range(2):
                        nc.tensor.matmul(out=psx[32*smp:32*smp+32, 64*lr:64*lr+64],
                                         lhsT=xT_all[0:65, 32*bs:32*bs+32],
                                         rhs=gatw[:, 2*L+lr, :], start=True, stop=True,
                                         tile_position=(0, 32*smp))
                xl = R(15)[:, 128*g:128*g+64]
                nc.vector.tensor_copy(xl, psx[:, 0:64])
                xr = R(15)[:, 128*g+64:128*g+128]
                nc.vector.tensor_copy(xr, psx[:, 64:128])
                xrf = (R(16) if g % 2 == 0 else R(7))[0:4, :]
                for smp in range(4):
                    nc.sync.dma_start(out=xrf[smp:smp+1, :].rearrange("p (d h) -> p d h", d=32),
                                      in_=xr[32*smp:32*smp+32, :])
                e3 = R(17 + g % 2)
                for q in range(4):
                    psxb = ps_tile([128, 512], f"psxb{L}{g}{q}")
                    nc.tensor.matmul(out=psxb[:], lhsT=od2[:], rhs=xrf[:, 512*q:512*q+512],
                                     start=True, stop=True)
                    e1 = R(19)[:, 1024*(g % 2):1024*(g % 2)+512]
                    nc.vector.tensor_add(
                        e1.rearrange("p (d h) -> p d h", d=8), psxb[:].rearrange("p (d h) -> p d h", d=8),
                        xl[:, None, :].broadcast_to([128, 8, 64]))
                    e2 = R(19)[:, 1024*(g % 2)+512:1024*(g % 2)+1024]
                    nc.scalar.activation(e2, e1, AF.Lrelu, alpha=0.2)
                    nc.vector.tensor_mul(
                        e3[:, 512*q:512*q+512].rearrange("p (d h) -> p d h", d=8),
                        e2.rearrange("p (d h) -> p d h", d=8),
                        gatt[:, L, :][:, None, :].broadcast_to([128, 8, 64]))
                RL = R(26) if g % 2 == 0 else R(13)
                lg = RL.bitcast(f32)[:, 208:240]
                nc.vector.tensor_reduce(lg, e3[:].rearrange("p (d h) -> p d h", d=32),
                                        axis=mybir.AxisListType.X, op=mybir.AluOpType.add)
                elg = RL[:, 320:352]
                nc.scalar.activation(elg, lg, AF.Exp)
                psd2 = ps_tile([4, 32], f"psd2{L}{g}")
                nc.tensor.matmul(out=psd2[:], lhsT=od[:], rhs=elg, start=True, stop=True)
                rd2f = RL.bitcast(f32)[0:4, 304:336]
                nc.vector.reciprocal_approx_fast(out=rd2f, in_=psd2[:])
                rd2 = RL[0:4, 384:416]
                nc.scalar.copy(rd2, rd2f)
                psb2 = ps_tile([128, 32], f"psb2{L}{g}")
                nc.tensor.matmul(out=psb2[:], lhsT=od2[:], rhs=rd2, start=True, stop=True)
                alp = RL[:, 352:384]
                nc.vector.tensor_mul(alp, elg, psb2[:])
                psg = ps_tile([128, 64], f"psg{L}{g}")
                for smp in range(4):
                    nc.tensor.matmul(out=psg[32*smp:32*smp+32, :],
                                     lhsT=alp[32*smp:32*smp+32, :],
                                     rhs=xl[32*smp:32*smp+32, :],
                                     start=True, stop=True,
                                     tile_position=(32*smp, 32*smp))
                gb = RL.bitcast(f32)[:, 240:304]
                nc.vector.tensor_add(gb, psg[:], gbias[:, L, :])
                nc.scalar.activation(R(gout_off)[:, 64*g:64*g+64], gb, AF.Tanh)

        gat_layer(0, xnT, 20)
        for gg in range(4):
            nc.sync.dma_start(out=att_pad[128*gg:128*gg+128, 64:128], in_=zpad)
            nc.scalar.dma_start(out=att_pad[128*gg:128*gg+128, 0:64],
                                in_=R(20)[:, 64*gg:64*gg+64])
        g0T = R(21)[:, 0:512]
        nc.sync.dma_start_transpose(g0T, att_pad[:, :])
        gat_layer(1, g0T, 22)
        if DEBUG:
            nc.sync.dma_start(out=dbg_g01[0], in_=R(20)[:, 0:64])
            nc.sync.dma_start(out=dbg_g01[1], in_=R(22)[:, 0:64])

        # ---- fusion ----
        for g in range(4):
            gs = R(21)[:, 512 + 64*g: 512 + 64*(g+1)]
            nc.vector.tensor_add(gs, R(20)[:, 64*g:64*g+64], R(22)[:, 64*g:64*g+64])
            nc.sync.dma_start(out=fus_nat[128*g:128*g+128, 0:64], in_=xn_st[g])
            nc.scalar.dma_start(out=fus_nat[128*g:128*g+128, 64:128], in_=gs)
        fusT = R(23)[:, 0:512]
        nc.sync.dma_start_transpose(fusT, fus_nat[:, :])

        # ---- caps ----
        fwt = sb.tile([65, 32], f16, name="fwt", tag="fwt")
        nc.sync.dma_start(fwt[:], fw_d[:, :])
        onesrow = R(9)[0:1, 0:512]
        nc.vector.memset(onesrow, 1.0)
        nc.sync.dma_start(out=caps_pad[64:65, :], in_=onesrow)
        RAs = [R(24), R(0), R(5), R(7)]
        RBs = [R(25), R(1), R(6), R(8)]
        for mt in range(16):
            RA = RAs[mt % 4]
            RB = RBs[mt % 4]
            wc = R(26)[:, 128*(mt % 4):128*(mt % 4)+128]
            nc.sync.dma_start(wc, wc_d[mt])
            pscap = ps_tile([128, 512], f"pscap{mt}")
            nc.tensor.matmul(out=pscap[:], lhsT=wc, rhs=fusT, start=True, stop=True)
            P = RA[:, 0:512]
            nc.vector.tensor_copy(P, pscap[:])
            o0 = sb.tile([128, 16], f32, name=f"o0{mt}", tag="o0", bufs=4)
            nc.vector.tensor_reduce(o0[:], P.rearrange("p (b c) -> p b c", b=16),
                                    axis=mybir.AxisListType.X, op=mybir.AluOpType.add)
            o0s = sb.tile([128, 16], f16, name=f"o0s{mt}", tag="o0s", bufs=4)
            nc.vector.tensor_scalar_mul(o0s[:], o0[:], 1.0/32.0)
            Lcur = RA[:, 512:1024]
            nc.vector.tensor_mul(Lcur.rearrange("p (b c) -> p b c", b=16),
                                 P.rearrange("p (b c) -> p b c", b=16),
                                 o0s[:][:, :, None].broadcast_to([128, 16, 32]))
            out_prev = o0s
            for it in (1, 2):
                Et = RA[:, 1024:1536]
                nc.scalar.activation(Et, Lcur, AF.Exp)
                EP = RA[:, 1536:2048]
                nc.gpsimd.tensor_mul(EP, Et, P)
                psdc = ps_tile([4, 512], f"psdc{mt}{it}")
                nc.tensor.matmul(out=psdc[:], lhsT=od[:], rhs=Et, start=True, stop=True)
                rdcf = (R(2 + mt % 2) if mt % 4 < 2 else R(9 + mt % 2)).bitcast(f32)[0:4, 512*(it-1):512*(it-1)+512]
                nc.vector.reciprocal_approx_fast(out=rdcf, in_=psdc[:])
                rdc = (R(27) if mt % 4 < 2 else R(11))[0:4, 512*(2*(mt % 2) + it - 1):512*(2*(mt % 2) + it - 1)+512]
                nc.gpsimd.tensor_copy(rdc, rdcf)
                psbc = ps_tile([128, 512], f"psbc{mt}{it}")
                nc.tensor.matmul(out=psbc[:], lhsT=od2[:], rhs=rdc, start=True, stop=True)
                pp = RB[:, 0:512]
                nc.vector.tensor_mul(pp, EP, psbc[:])
                oo = sb.tile([128, 16], f32, name=f"oo{mt}{it}", tag="o0", bufs=4)
                nc.vector.tensor_reduce(oo[:], pp.rearrange("p (b c) -> p b c", b=16),
                                        axis=mybir.AxisListType.X, op=mybir.AluOpType.add)
                oos = sb.tile([128, 16], f16, name=f"oos{mt}{it}", tag="oos", bufs=4)
                nc.gpsimd.tensor_copy(oos[:], oo[:])
                out_prev = oos
                if it == 1:
                    m2 = RB[:, 512:1024]
                    nc.vector.tensor_mul(m2.rearrange("p (b c) -> p b c", b=16),
                                         P.rearrange("p (b c) -> p b c", b=16),
                                         oos[:][:, :, None].broadcast_to([128, 16, 32]))
                    L2 = RB[:, 1024:1536]
                    nc.vector.tensor_add(L2, Lcur, m2)
                    Lcur = L2
            tc_t = sb.tile([128, 16], f16, name=f"tc{mt}", tag="tc", bufs=4)
            nc.scalar.activation(tc_t[:], out_prev[:], AF.Tanh)
            if DEBUG:
                nc.sync.dma_start(out=dbg_caps[mt], in_=tc_t[:])
            for l_loc in range(4):
                eng4 = nc.sync if l_loc % 2 == 0 else nc.gpsimd
                eng4.dma_start(
                    out=caps_pad[4*mt+l_loc].rearrange("(o s) -> o s", o=32),
                    in_=tc_t[32*l_loc:32*l_loc+32, :])
        capsT = R(23)[:, 512:1024]
        nc.sync.dma_start(capsT[0:65, :], caps_pad[0:65, :])
        psf = ps_tile([32, 512], "psf")
        nc.tensor.matmul(out=psf[:], lhsT=fwt[:], rhs=capsT[0:65, :], start=True, stop=True)
        fin = R(25)[0:32, 1024:1536]
        nc.scalar.activation(fin, psf[:], AF.Tanh)
        nc.sync.dma_start(out=out_d.ap().rearrange("dd o s -> dd (o s)"),
                          in_=fin)

        for p_ in reversed(ctxs):
            p_.__exit__(None, None, None)
    nc.compile()
    return nc


# ===================== host side =====================
_NC_CACHE = {}

def _get_program():
    if "prog" not in _NC_CACHE:
        _NC_CACHE["prog"] = build_program()
    return _NC_CACHE["prog"]


def _prep_inputs(inputs):
    X = np.asarray(inputs["inputs"], np.float32)
    X = np.nan_to_num(X, nan=0.0, posinf=1.0)
    ei = np.asarray(inputs["edge_index"])
    s = np.repeat(np.arange(D), D); t = np.tile(np.arange(D), D)
    off = (np.arange(B) * D)[:, None]
    exp_ei = np.stack([(s[None] + off).reshape(-1), (t[None] + off).reshape(-1)]).astype(ei.dtype)
    assert np.array_equal(ei, exp_ei), "edge_index mismatch vs block-diagonal pattern"

    # [B, T, F] -> [B, 128 part, KT*32] with partition-major contiguous layout
    xT = np.ascontiguousarray(
        np.swapaxes(X, 1, 2).reshape(B, KT, 128, 32).transpose(0, 2, 1, 3)
    ).reshape(B, 128, KT*32).astype(np.float16)

    wih0p = _gate_cols(np.asarray(inputs["Wih0"], np.float32)) * S
    wih1p = _gate_cols(np.asarray(inputs["Wih1"], np.float32)) * S
    whh0p = _gate_cols(np.asarray(inputs["Whh0"], np.float32)) * S
    whh1p = _gate_cols(np.asarray(inputs["Whh1"], np.float32)) * S
    whh0_dev = np.ascontiguousarray(whh0p.reshape(KT, 128, 6144)).astype(ml_dtypes.float8_e4m3)
    whh1_dev = np.ascontiguousarray(whh1p.reshape(KT, 128, 6144)).astype(ml_dtypes.float8_e4m3)

    def bias_strip(bih, bhh):
        b = np.zeros(6144, np.float32)
        for q in range(8):
            hds = PERM[q*256:(q+1)*256]
            b[q*768+0*256: q*768+1*256] = bih[0*2048 + hds] + bhh[0*2048 + hds]
            b[q*768+1*256: q*768+2*256] = bih[1*2048 + hds] + bhh[1*2048 + hds]
            b[q*768+2*256: q*768+3*256] = bih[2*2048 + hds]
        return b * S
    bih0 = np.asarray(inputs["bih0"], np.float32); bhh0 = np.asarray(inputs["bhh0"], np.float32)
    bih1 = np.asarray(inputs["bih1"], np.float32); bhh1 = np.asarray(inputs["bhh1"], np.float32)
    bs0 = bias_strip(bih0, bhh0).astype(np.float16)
    bs1 = bias_strip(bih1, bhh1).astype(np.float16)

    def bhn_bcast(bhh):
        outb = np.zeros((128, 2, 256), np.float32)
        for Hh in range(2):
            for j in range(4):
                hds = PERM[(Hh*4+j)*256:(Hh*4+j)*256+256]
                outb[32*j:32*j+32, Hh, :] = bhh[2*2048 + hds][None, :]
        return (outb * S).astype(np.float16)

    eye = np.eye(128, dtype=np.float16)
    A_w = np.asarray(inputs["A_w"], np.float32); A_b = np.asarray(inputs["A_b"], np.float32)
    awt = np.tile(A_w.T.astype(np.float16), (4, 1))
    ab = np.tile(A_b, 4)[:, None].astype(np.float32)
    od = np.zeros((128, 4), np.float16)
    for gq in range(4):
        od[32*gq:32*gq+32, gq] = 1.0
    od2 = np.ascontiguousarray(od.T)

    gw = np.zeros((65, 4, 64), np.float16)
    for L, pfx in enumerate(["g0", "g1"]):
        for lr, nm in enumerate(["l", "r"]):
            gw[0:64, 2*L+lr] = np.asarray(inputs[f"{pfx}_W{nm}"], np.float32).T.astype(np.float16)
            gw[64, 2*L+lr] = np.asarray(inputs[f"{pfx}_b{nm}"], np.float32).astype(np.float16)
    gat_att = np.zeros((2, 128, 64), np.float16)
    gat_bias = np.zeros((2, 128, 64), np.float16)
    for L, pfx in enumerate(["g0", "g1"]):
        gat_att[L] = np.tile(np.asarray(inputs[f"{pfx}_att"], np.float32), (128, 1)).astype(np.float16)
        gat_bias[L] = np.tile(np.asarray(inputs[f"{pfx}_bias"], np.float32), (128, 1)).astype(np.float16)

    Wc = np.asarray(inputs["W_caps"], np.float32)
    wc_t = np.zeros((16, 128, 128), np.float16)
    for mt in range(16):
        for l_loc in range(4):
            l = 4*mt + l_loc
            wc_t[mt, :, 32*l_loc:32*l_loc+32] = Wc[:, l, :].T.astype(np.float16)
    fw = np.zeros((65, 32), np.float16)
    fw[0:64] = np.asarray(inputs["F_w"], np.float32).T.astype(np.float16)
    fw[64] = np.asarray(inputs["F_b"], np.float32).astype(np.float16)

    common = dict(xT=xT, whh0=whh0_dev, whh1=whh1_dev,
                  bhn0=bhn_bcast(bhh0), bhn1=bhn_bcast(bhh1), eye=eye,
                  awt=awt, ab=ab, od=od, od2=od2, gw=gw, gat_att=gat_att,
                  gat_bias=gat_bias, wc=wc_t, fw=fw)
    in_maps = []
    for r in range(NC):
        sel = np.zeros((128, SH), np.float16)
        for i in range(SH):
            sel[SH*r + i, i] = 1.0
        m = dict(common)
        m["wih0"] = np.ascontiguousarray(wih0p[:, 768*r:768*r+768].astype(np.float16).reshape(KT, 128, 768))
        m["wih1"] = np.ascontiguousarray(
            wih1p[:, 768*r:768*r+768].astype(ml_dtypes.float8_e4m3).reshape(KT, 128, 768))
        m["b0"] = bs0[768*r:768*r+768][None, :].copy()
        m["b1"] = bs1[768*r:768*r+768][None, :].copy()
        m["sel"] = sel
        in_maps.append(m)
    return in_maps


def kernel(**inputs):
    in_maps = _prep_inputs(inputs)
    nc = _get_program()
    res = run_bass_kernel_spmd(nc, in_maps, list(range(NC)))
    out = np.concatenate([res.results[r]["out"].transpose(2, 1, 0) for r in range(NC)], axis=0)
    return out.astype(np.float32)


if __name__ == "__main__":
    t0 = time.time()
    build_program()
    print("build+compile", time.time() - t0)
